# revision 1
# baseline (speedup 1.0000x reference)
"""Expert-parallel MoE kernel for Trainium2 (8 NeuronCores).

Problem: top-2 MoE, N=8192 tokens, D=1024, H=4096, E=8 experts.
Strategy (expert parallel):
  - Host: compute gating (logits -> top-k -> softmax) exactly as the
    reference does (CPU jax, fp32), dispatch tokens to their experts.
  - Core e holds expert e's weights; it runs a 2-layer MLP over the
    tokens routed to it (padded to a fixed capacity C), plus the
    combine() row-renormalization:
        y = (relu(x @ w1 + b1) @ w2 + b2)
        y_scaled = y * (gate * ||x||) / (||y|| + 1e-8)
  - Host: scatter-add per-expert outputs back to the [N, D] result.

Device kernel (per core, bf16 matmuls, fp32 PSUM accumulation):
  Token blocks of <=512. Layer 1 computes hT [H, R] (H on partitions) by
  streaming w1 in per-h-tile chunks; layer 2 accumulates out[R, D] in
  PSUM over the 32 H-tiles with w2 resident in SBUF. Epilogue: +b2,
  row sum-of-squares (ACT Square with accum_out), sqrt, reciprocal,
  final scale, DMA out.

  Inputs are pre-tiled on the host so every DMA chunk is contiguous per
  partition (2-8KB runs; untiled layouts measured only ~138GB/s):
    xT  [P, n_k*C]        xT[p, n_k*B + k*R + j] = x[tok B+j, k*128+p]
    w1  [P, n_h, n_k, P]  w1[p, h, k, j] = w1[k*128+p, h*128+j]
    w2  [P, n_h, D]       w2[p, h, d]    = w2[h*128+p, d]
  DMA queue discipline: x/w1 stream on the sync-engine HWDGE queue; the
  8MB w2 load on the scalar-engine queue (delayed behind the first
  stream chunk); y outputs on the gpsimd SWDGE queue (an engine-FIFO
  DMA trigger on ACT would block layer-1 relu evacuation).
"""

import os
import sys

import numpy as np

if "/opt/trn_rl_repo" not in sys.path:
    sys.path.insert(0, "/opt/trn_rl_repo")

import ml_dtypes

N, D, H, E = 8192, 1024, 4096, 8
P = 128
BLK = 512  # max token block
NK = D // P   # 8
NH = H // P   # 32
BF16 = ml_dtypes.bfloat16

_nc_cache = {}


def _blocks_for(C):
    # Full blocks first, small remainder last: a leading small block would
    # make layer 1 consume w1 at ~580GB/s (N=128 matmuls) and stall on HBM;
    # as the last block its layer 1 prefetches under the previous block's
    # layer 2 instead.
    blocks = []
    off = 0
    while off < C:
        r = min(BLK, C - off)
        blocks.append((off, r))
        off += r
    return blocks


def _tile_w1(w1e):
    """[D, H] fp32 -> [P, NH, NK, P] bf16 with w1t[p,h,k,j] = w1e[k*P+p, h*P+j]."""
    return np.ascontiguousarray(
        w1e.reshape(NK, P, NH, P).transpose(1, 2, 0, 3).astype(BF16))


def _tile_w2(w2e):
    """[H, D] fp32 -> [P, NH, D] bf16 with w2t[p,h,d] = w2e[h*P+p, d]."""
    return np.ascontiguousarray(
        w2e.reshape(NH, P, D).transpose(1, 0, 2).astype(BF16))


def _tile_xT(xg, C):
    """[C, D] fp32 (padded) -> [P, NK*C] bf16, per-block [k, j] segments."""
    out = np.zeros((P, NK * C), BF16)
    for B, R in _blocks_for(C):
        seg = xg[B:B + R].T.reshape(NK, P, R).transpose(1, 0, 2)
        out[:, NK * B:NK * (B + R)] = seg.reshape(P, NK * R)
    return out


def _build_nc(C):
    """Build the per-core Bass program for capacity C (multiple of 128)."""
    from contextlib import ExitStack

    import concourse.bass as bass
    import concourse.mybir as mybir
    import concourse.tile as tile
    from concourse import bacc

    f32 = mybir.dt.float32
    bf16 = mybir.dt.bfloat16
    AF = mybir.ActivationFunctionType

    nc = bacc.Bacc(trn_type="TRN2", num_devices=E)
    xT = nc.dram_tensor("xT", [P, NK * C], bf16, kind="ExternalInput")
    w1 = nc.dram_tensor("w1", [P, NH, NK, P], bf16, kind="ExternalInput")
    b1 = nc.dram_tensor("b1", [P, NH], f32, kind="ExternalInput")
    w2 = nc.dram_tensor("w2", [P, NH, D], bf16, kind="ExternalInput")
    b2 = nc.dram_tensor("b2", [D], f32, kind="ExternalInput")
    sc = nc.dram_tensor("sc", [P, C // P], f32, kind="ExternalInput")
    y = nc.dram_tensor("y", [C, D], f32, kind="ExternalOutput")

    y_t = y.ap().rearrange("(o p) d -> p o d", p=P)

    blocks = _blocks_for(C)

    with tile.TileContext(nc) as tc, ExitStack() as ctx:
        singles = ctx.enter_context(tc.tile_pool(name="singles", bufs=1))
        xpool = ctx.enter_context(tc.tile_pool(name="xpool", bufs=2))
        w1pool = ctx.enter_context(tc.tile_pool(name="w1pool", bufs=6))
        hpool = ctx.enter_context(tc.tile_pool(name="hpool", bufs=2))
        stpool = ctx.enter_context(tc.tile_pool(name="stpool", bufs=1))
        sqpool = ctx.enter_context(tc.tile_pool(name="sqpool", bufs=2))
        smpool = ctx.enter_context(tc.tile_pool(name="smpool", bufs=4))
        psh = ctx.enter_context(tc.tile_pool(name="psh", bufs=4, space="PSUM"))
        pso = ctx.enter_context(tc.tile_pool(name="pso", bufs=2, space="PSUM"))

        # --- preamble: constants ---
        b1_sb = singles.tile([P, NH], f32)
        nc.gpsimd.dma_start(out=b1_sb, in_=b1.ap())
        b2_sb = singles.tile([P, D], f32)
        b2_bcast = bass.AP(tensor=b2.ap().tensor, offset=b2.ap().offset,
                           ap=[[0, P], *b2.ap().ap])
        nc.gpsimd.dma_start(out=b2_sb, in_=b2_bcast)
        sc_sb = singles.tile([P, C // P], f32)
        nc.gpsimd.dma_start(out=sc_sb, in_=sc.ap())
        # w2 is loaded in 1MB chunks spread through block-0's layer 1 (the
        # triggers sit between relus in the ACT FIFO), so it neither hogs
        # HBM during startup nor misses its first layer-2 use.
        w2_sb = singles.tile([P, NH, D], bf16)

        for (B, R) in blocks:
            m_tiles = R // P
            xt = xpool.tile([P, NK, BLK], bf16, tag="xt", name="xt")[:, :, :R]
            nc.sync.dma_start(
                out=xt,
                in_=xT.ap()[:, NK * B:NK * (B + R)].rearrange(
                    "p (k j) -> p k j", k=NK))

            # --- layer 1: hT[h, tok] = relu(x @ w1 + b1), H on partitions ---
            hT = hpool.tile([P, NH, BLK], bf16, tag="hT", name="hT")[:, :, :R]
            for h in range(NH):
                w1c = w1pool.tile([P, NK, P], bf16, tag="w1c")
                nc.sync.dma_start(out=w1c, in_=w1.ap()[:, h])
                if B == 0 and h % 4 == 3:
                    # w2 rows ride the same FIFO queue, paced between the
                    # w1 chunks so they never starve the layer-1 stream.
                    nc.sync.dma_start(out=w2_sb[:, h - 3:h + 1, :],
                                      in_=w2.ap()[:, h - 3:h + 1, :])
                ps = psh.tile([P, BLK], f32, tag="ph", name="ph")[:, :R]
                for k in range(NK):
                    nc.tensor.matmul(
                        ps,
                        lhsT=w1c[:, k, :],
                        rhs=xt[:, k, :],
                        start=(k == 0),
                        stop=(k == NK - 1),
                    )
                nc.scalar.activation(
                    out=hT[:, h, :], in_=ps, func=AF.Relu,
                    bias=b1_sb[:, h:h + 1], scale=1.0,
                )

            # --- layer 2: out[tok, D] accumulated over h; epilogue ---
            stage = stpool.tile([P, BLK // P, D], f32, tag="stage",
                                name="stage")[:, :m_tiles, :]
            q = smpool.tile([P, BLK // P], f32, tag="q", name="q")[:, :m_tiles]
            for m in range(m_tiles):
                po = pso.tile([P, D], f32, tag="po")
                for h in range(NH):
                    for n2 in range(2):
                        nc.tensor.matmul(
                            po[:, n2 * 512:(n2 + 1) * 512],
                            lhsT=hT[:, h, m * P:(m + 1) * P],
                            rhs=w2_sb[:, h, n2 * 512:(n2 + 1) * 512],
                            start=(h == 0),
                            stop=(h == NH - 1),
                        )
                # stage = out + b2 ; q[:, m] = sum(stage^2)
                nc.vector.tensor_add(out=stage[:, m, :], in0=po, in1=b2_sb)
                sq = sqpool.tile([P, D], f32, tag="sq")
                nc.scalar.activation(
                    out=sq, in_=stage[:, m, :], func=AF.Square,
                    accum_out=q[:, m:m + 1],
                )

            # f = sc / (sqrt(q) + 1e-8); y = stage * f
            qs = smpool.tile([P, BLK // P], f32, tag="qs", name="qs")[:, :m_tiles]
            nc.scalar.activation(out=qs, in_=q, func=AF.Sqrt)
            nc.vector.tensor_scalar_add(out=qs, in0=qs, scalar1=1e-8)
            nc.vector.reciprocal(out=qs, in_=qs)
            f = smpool.tile([P, BLK // P], f32, tag="f", name="f")[:, :m_tiles]
            nc.vector.tensor_mul(out=f, in0=qs,
                                 in1=sc_sb[:, B // P:B // P + m_tiles])
            for m in range(m_tiles):
                nc.vector.tensor_scalar_mul(
                    out=stage[:, m, :], in0=stage[:, m, :],
                    scalar1=f[:, m:m + 1],
                )
                nc.gpsimd.dma_start(out=y_t[:, B // P + m, :],
                                    in_=stage[:, m, :])

    nc.compile()
    return nc


def _get_nc(C):
    if C not in _nc_cache:
        _nc_cache[C] = _build_nc(C)
    return _nc_cache[C]


LAST_EXEC_NS = None
LAST_TRACE = None


def _install_axon_ntff_hook():
    """Register antenv.axon_hooks shim driving NTFF capture via the axon .so.

    The agent image's antenv package lacks axon_hooks, so concourse's
    trace=True path degrades. Replicates trn_boot._ntff_profile_via_ctypes.
    """
    import contextlib
    import ctypes
    import types

    if "antenv.axon_hooks" in sys.modules:
        return
    lib = ctypes.CDLL("/opt/axon/libaxon_pjrt.so")
    if not hasattr(lib, "axon_start_nrt_profile"):
        return
    lib.axon_start_nrt_profile.argtypes = [ctypes.POINTER(ctypes.c_int64),
                                           ctypes.c_size_t]
    lib.axon_start_nrt_profile.restype = ctypes.c_int64
    lib.axon_stop_nrt_profile.argtypes = [ctypes.c_char_p]
    lib.axon_stop_nrt_profile.restype = ctypes.c_int64

    @contextlib.contextmanager
    def _hook(output_dir, device_ids):
        import jax
        jax.devices()
        if device_ids:
            ids = (ctypes.c_int64 * len(device_ids))(*device_ids)
            rc = lib.axon_start_nrt_profile(ids, len(device_ids))
        else:
            rc = lib.axon_start_nrt_profile(None, 0)
        if rc != 0:
            raise RuntimeError(f"axon_start_nrt_profile rc={rc}")
        try:
            yield
        finally:
            n = lib.axon_stop_nrt_profile(str(output_dir).encode())
            print(f"ntff capture: {n} file(s) -> {output_dir}", file=sys.stderr)

    mod = types.ModuleType("antenv.axon_hooks")
    mod.get_axon_ntff_profile_hook = lambda: _hook
    sys.modules["antenv.axon_hooks"] = mod
    import antenv
    antenv.axon_hooks = mod


def _gating(x, w_gate, k):
    """Top-k gating computed exactly like the reference (CPU jax, fp32)."""
    import jax
    import jax.numpy as jnp

    cpu = jax.devices("cpu")[0]
    with jax.default_device(cpu):
        xj = jnp.asarray(x)
        logits = xj @ jnp.asarray(w_gate)
        top_vals, top_idx = jax.lax.top_k(logits, k)
        top_gates = jax.nn.softmax(top_vals, axis=-1)
        init_norm = jnp.linalg.norm(xj, axis=-1)
        return (np.asarray(top_idx), np.asarray(top_gates, np.float32),
                np.asarray(init_norm, np.float32))


def kernel(x, w_gate, w1, b1, w2, b2, k):
    from concourse.bass_utils import run_bass_kernel_spmd

    x = np.asarray(x, np.float32)
    w_gate = np.asarray(w_gate, np.float32)
    w1 = np.asarray(w1, np.float32)
    b1 = np.asarray(b1, np.float32)
    w2 = np.asarray(w2, np.float32)
    b2 = np.asarray(b2, np.float32)
    k = int(np.asarray(k))
    n, d = x.shape
    e = w_gate.shape[1]

    top_idx, top_gates, init_norm = _gating(x, w_gate, k)

    idxs, scs = [], []
    for ei in range(e):
        tok, slot = np.nonzero(top_idx == ei)
        idxs.append(tok)
        scs.append(top_gates[tok, slot] * init_norm[tok])

    maxc = max(len(t) for t in idxs)
    C = max(((maxc + P - 1) // P) * P, P)
    nc = _get_nc(C)

    in_maps = []
    for ei in range(e):
        tok = idxs[ei]
        xg = np.zeros((C, d), np.float32)
        xg[:len(tok)] = x[tok]
        sce = np.zeros((C,), np.float32)
        sce[:len(tok)] = scs[ei]
        sce = np.ascontiguousarray(sce.reshape(C // P, P).T)
        in_maps.append({
            "xT": _tile_xT(xg, C),
            "w1": _tile_w1(w1[ei]),
            "b1": np.ascontiguousarray(b1[ei].reshape(NH, P).T),
            "w2": _tile_w2(w2[ei]),
            "b2": np.ascontiguousarray(b2[ei]),
            "sc": sce,
        })

    trace = bool(int(os.environ.get("MOE_TRACE", "0")))
    kwargs = {}
    if trace:
        _install_axon_ntff_hook()
        tdir = os.environ.get("MOE_TRACE_DIR")
        if tdir:
            os.makedirs(tdir, exist_ok=True)
            kwargs["tmpdir"] = tdir
        kwargs["trace_cores"] = [0]
    res = run_bass_kernel_spmd(
        nc, in_maps, core_ids=list(range(e)), trace=trace, **kwargs,
    )
    global LAST_EXEC_NS, LAST_TRACE
    LAST_EXEC_NS = res.exec_time_ns
    LAST_TRACE = res.instructions_and_trace
    if res.exec_time_ns is not None:
        print(f"HW exec time: {res.exec_time_ns} ns", file=sys.stderr)

    y = np.zeros((n, d), np.float32)
    for ei in range(e):
        tok = idxs[ei]
        y[tok] += res.results[ei]["y"][:len(tok)]
    return y



# revision 3
# speedup vs baseline: 1.0800x; 1.0800x over previous
"""Expert-parallel mixed-precision MoE kernel for Trainium2 (8 NeuronCores).

Problem: top-2 MoE, N=8192 tokens, D=1024, H=4096, E=8 experts.
Strategy (expert parallel, per-token mixed precision):
  - Host: compute gating (logits -> top-k -> softmax) exactly as the
    reference does (CPU jax, fp32), dispatch tokens to their experts.
  - Core e holds expert e's tokens, split into two tiers by combine
    weight g*||x||: the top C_HI=1664 run a bf16 pipeline, the rest
    (<=C_LO=512) run an fp8-e4m3 DoubleRow pipeline (2x matmul rate,
    measured 216ns for a 256-deep x 512-wide MM).  Output combine
    rescale:  y = (mlp(x)) * (gate * ||x||) / (||mlp(x)|| + 1e-8).
    fp8 error only hits low-gate tokens; exact rel err ~1.5e-2.
  - Host: scatter-add per-expert outputs back to the [N, D] result.

Device kernel (per core, fp32 PSUM accumulation):
  Token blocks: hi [512, 384, 384, 384] bf16, lo [512] fp8.  Layer 1
  computes hT [H, R] (H on partitions) by streaming w1 per h-tile;
  layer 2 accumulates out[R, D] in PSUM over h with w2 resident in
  SBUF (bf16 64KB + fp8 32KB per partition). Epilogue: +b2, row
  sum-of-squares (ACT Square accum_out), sqrt, reciprocal, scale, DMA.
  fp8 scales (x*16, w1*1024, h*32, w2*1024) fold into the ACT relu
  scale/bias and the epilogue's b2/eps constants; the final y is
  exact in fp32 (stage*f cancels the 32768x product scale).

  DMA: block-0 xt is loaded in four 2-k-tile chunks interleaved with
  the first h-group's matmuls so MM#1 starts ~9us instead of ~15us.
  w2/w2_f8 ride the sync HWDGE FIFO paced between w1 chunks of the
  first two blocks; y outputs go on the gpsimd SWDGE queue except the
  last m-tile (sync, cheaper completion for the kernel tail).
"""

import os
import sys

import numpy as np

if "/opt/trn_rl_repo" not in sys.path:
    sys.path.insert(0, "/opt/trn_rl_repo")

import ml_dtypes

N, D, H, E = 8192, 1024, 4096, 8
P = 128
NK = D // P   # 8
NH = H // P   # 32
BF16 = ml_dtypes.bfloat16
F8 = ml_dtypes.float8_e4m3

C_HI, C_LO = 1664, 512
HI_BLOCKS = [512, 384, 384, 384]
LO_BLOCKS = [512]
C_TOT = C_HI + C_LO
S_X, S_W1, S_H, S_W2 = 16.0, 1024.0, 32.0, 1024.0
S_L1 = S_X * S_W1            # layer-1 psum scale
S_OUT = S_H * S_W2           # layer-2 psum scale

assert sum(HI_BLOCKS) == C_HI and sum(LO_BLOCKS) == C_LO

_nc_cache = {}


def _tile_w1(w1e, dt):
    """[D, H] fp32 -> [P, NH, NK, P] with w1t[p,h,k,j] = w1e[k*P+p, h*P+j]."""
    return np.ascontiguousarray(
        w1e.reshape(NK, P, NH, P).transpose(1, 2, 0, 3).astype(dt))


def _tile_w2(w2e, dt):
    """[H, D] fp32 -> [P, NH, D] with w2t[p,h,d] = w2e[h*P+p, d]."""
    return np.ascontiguousarray(
        w2e.reshape(NH, P, D).transpose(1, 0, 2).astype(dt))


def _tile_xT(xg, blocks, dt):
    """[C, D] fp32 (padded) -> [P, NK*C], per-block [k, j] segments."""
    C = xg.shape[0]
    out = np.zeros((P, NK * C), dt)
    B = 0
    for R in blocks:
        seg = xg[B:B + R].T.reshape(NK, P, R).transpose(1, 0, 2)
        out[:, NK * B:NK * (B + R)] = seg.reshape(P, NK * R)
        B += R
    return out


def _q8(a, scale):
    return np.clip(np.asarray(a, np.float32) * scale, -240, 240).astype(F8)


def _build_nc():
    from contextlib import ExitStack

    import concourse.bass as bass
    import concourse.mybir as mybir
    import concourse.tile as tile
    from concourse import bacc

    f32 = mybir.dt.float32
    bf16 = mybir.dt.bfloat16
    f8 = mybir.dt.float8e4
    AF = mybir.ActivationFunctionType
    DR = mybir.MatmulPerfMode.DoubleRow

    nc = bacc.Bacc(trn_type="TRN2", num_devices=E)
    xT = nc.dram_tensor("xT", [P, NK * C_HI], bf16, kind="ExternalInput")
    xT8 = nc.dram_tensor("xT8", [P, NK * C_LO], f8, kind="ExternalInput")
    w1 = nc.dram_tensor("w1", [P, NH, NK, P], bf16, kind="ExternalInput")
    w18 = nc.dram_tensor("w18", [P, NH, NK, P], f8, kind="ExternalInput")
    b1 = nc.dram_tensor("b1", [P, NH], f32, kind="ExternalInput")
    b1s = nc.dram_tensor("b1s", [P, NH], f32, kind="ExternalInput")
    w2 = nc.dram_tensor("w2", [P, NH, D], bf16, kind="ExternalInput")
    w28 = nc.dram_tensor("w28", [P, NH, D], f8, kind="ExternalInput")
    b2 = nc.dram_tensor("b2", [D], f32, kind="ExternalInput")
    b2s = nc.dram_tensor("b2s", [D], f32, kind="ExternalInput")
    sc = nc.dram_tensor("sc", [P, C_TOT // P], f32, kind="ExternalInput")
    y = nc.dram_tensor("y", [C_TOT, D], f32, kind="ExternalOutput")

    y_t = y.ap().rearrange("(o p) d -> p o d", p=P)

    with tile.TileContext(nc) as tc, ExitStack() as ctx:
        singles = ctx.enter_context(tc.tile_pool(name="singles", bufs=1))
        xpool = ctx.enter_context(tc.tile_pool(name="xpool", bufs=2))
        x0pool = ctx.enter_context(tc.tile_pool(name="x0pool", bufs=1))
        w1pool = ctx.enter_context(tc.tile_pool(name="w1pool", bufs=5))
        hpool = ctx.enter_context(tc.tile_pool(name="hpool", bufs=1))
        stpool = ctx.enter_context(tc.tile_pool(name="stpool", bufs=1))
        sqpool = ctx.enter_context(tc.tile_pool(name="sqpool", bufs=1))
        smpool = ctx.enter_context(tc.tile_pool(name="smpool", bufs=4))
        psh = ctx.enter_context(tc.tile_pool(name="psh", bufs=3, space="PSUM"))
        pso = ctx.enter_context(tc.tile_pool(name="pso", bufs=2, space="PSUM"))

        # --- constants (gpsimd SWDGE queue; small) ---
        b1_sb = singles.tile([P, NH], f32)
        nc.gpsimd.dma_start(out=b1_sb, in_=b1.ap())
        b1s_sb = singles.tile([P, NH], f32)
        nc.gpsimd.dma_start(out=b1s_sb, in_=b1s.ap())
        sc_sb = singles.tile([P, C_TOT // P], f32)
        nc.gpsimd.dma_start(out=sc_sb, in_=sc.ap())
        b2_sb = singles.tile([P, D], f32)
        b2_bcast = bass.AP(tensor=b2.ap().tensor, offset=b2.ap().offset,
                           ap=[[0, P], *b2.ap().ap])
        nc.gpsimd.dma_start(out=b2_sb, in_=b2_bcast)
        b2s_sb = singles.tile([P, D], f32)
        b2s_bcast = bass.AP(tensor=b2s.ap().tensor, offset=b2s.ap().offset,
                            ap=[[0, P], *b2s.ap().ap])
        nc.gpsimd.dma_start(out=b2s_sb, in_=b2s_bcast)
        # w2 / w2_f8 are paced between w1 chunks on the sync FIFO below.
        w2_sb = singles.tile([P, NH, D], bf16)
        w28_sb = singles.tile([P, NH, D], f8)

        blocks = ([("hi", B, R) for B, R in
                   zip(np.cumsum([0] + HI_BLOCKS[:-1]).tolist(), HI_BLOCKS)]
                  + [("lo", B, R) for B, R in
                     zip((C_HI + np.cumsum([0] + LO_BLOCKS[:-1])).tolist(),
                         LO_BLOCKS)])
        n_blk = len(blocks)

        for bi, (tier, B, R) in enumerate(blocks):
            m_tiles = (R + P - 1) // P
            lo = tier == "lo"
            first = bi == 0

            # --- fetch x block ---
            if first:
                # four 2-k-tile chunks, interleaved with the h=0 matmuls
                xchunks = [x0pool.tile([P, 2, 512], bf16, tag=f"x0_{c}",
                                       name=f"x0_{c}") for c in range(4)]
            elif lo:
                xt = xpool.tile([P, NK, 512], f8, tag="xt", name="xt8")[:, :, :R]
                nc.sync.dma_start(
                    out=xt,
                    in_=xT8.ap()[:, NK * (B - C_HI):NK * (B - C_HI + R)]
                    .rearrange("p (k j) -> p k j", k=NK))
            else:
                xt = xpool.tile([P, NK, 512], bf16, tag="xt", name="xt")[:, :, :R]
                nc.sync.dma_start(
                    out=xt,
                    in_=xT.ap()[:, NK * B:NK * (B + R)]
                    .rearrange("p (k j) -> p k j", k=NK))

            # --- layer 1: hT[h, tok] (H on partitions) ---
            hT = hpool.tile([P, NH, 512], bf16 if not lo else f8,
                            tag="hT", name=f"hT{bi}")[:, :, :R]
            for h in range(NH):
                w1c = w1pool.tile([P, NK, P], f8 if lo else bf16,
                                  tag="w1c", name=f"w1c{bi}_{h}")
                nc.sync.dma_start(out=w1c,
                                  in_=(w18 if lo else w1).ap()[:, h])
                if bi == 0 and h % 4 == 3:
                    # w2 rows ride the same FIFO, paced between w1 chunks
                    nc.sync.dma_start(out=w2_sb[:, h - 3:h + 1, :],
                                      in_=w2.ap()[:, h - 3:h + 1, :])
                if bi == 1 and h % 4 == 3:
                    nc.sync.dma_start(out=w28_sb[:, h - 3:h + 1, :],
                                      in_=w28.ap()[:, h - 3:h + 1, :])
                ps = psh.tile([P, 512], f32, tag="ph", name="ph")[:, :R]
                if first and h == 0:
                    for c in range(4):
                        nc.sync.dma_start(
                            out=xchunks[c],
                            in_=xT.ap()[:, NK * B + 2 * c * 512:
                                        NK * B + (2 * c + 2) * 512]
                            .rearrange("p (k j) -> p k j", k=2))
                        for k2 in range(2):
                            nc.tensor.matmul(
                                ps, lhsT=w1c[:, 2 * c + k2, :],
                                rhs=xchunks[c][:, k2, :],
                                start=(c == 0 and k2 == 0),
                                stop=(c == 3 and k2 == 1))
                elif first:
                    for c in range(4):
                        for k2 in range(2):
                            nc.tensor.matmul(
                                ps, lhsT=w1c[:, 2 * c + k2, :],
                                rhs=xchunks[c][:, k2, :],
                                start=(c == 0 and k2 == 0),
                                stop=(c == 3 and k2 == 1))
                elif lo:
                    for kp in range(NK // 2):
                        nc.tensor.matmul(
                            ps, lhsT=w1c[:, 2 * kp:2 * kp + 2, :],
                            rhs=xt[:, 2 * kp:2 * kp + 2, :],
                            start=(kp == 0), stop=(kp == NK // 2 - 1),
                            perf_mode=DR)
                else:
                    for k in range(NK):
                        nc.tensor.matmul(
                            ps, lhsT=w1c[:, k, :], rhs=xt[:, k, :],
                            start=(k == 0), stop=(k == NK - 1))
                nc.scalar.activation(
                    out=hT[:, h, :], in_=ps, func=AF.Relu,
                    bias=(b1s_sb if lo else b1_sb)[:, h:h + 1],
                    scale=(S_H / S_L1) if lo else 1.0,
                )

            # --- layer 2 + epilogue per m-tile ---
            stage = stpool.tile([P, 4, D], f32, tag="stage",
                                name="stage")[:, :m_tiles, :]
            q = smpool.tile([P, 4], f32, tag="q", name="q")[:, :m_tiles]
            for m in range(m_tiles):
                po = pso.tile([P, D], f32, tag="po")
                if lo:
                    for hp in range(NH // 2):
                        for n2 in range(2):
                            nc.tensor.matmul(
                                po[:, n2 * 512:(n2 + 1) * 512],
                                lhsT=hT[:, 2 * hp:2 * hp + 2,
                                        m * P:(m + 1) * P],
                                rhs=w28_sb[:, 2 * hp:2 * hp + 2,
                                           n2 * 512:(n2 + 1) * 512],
                                start=(hp == 0), stop=(hp == NH // 2 - 1),
                                perf_mode=DR)
                else:
                    for h in range(NH):
                        for n2 in range(2):
                            nc.tensor.matmul(
                                po[:, n2 * 512:(n2 + 1) * 512],
                                lhsT=hT[:, h, m * P:(m + 1) * P],
                                rhs=w2_sb[:, h, n2 * 512:(n2 + 1) * 512],
                                start=(h == 0), stop=(h == NH - 1))
                nc.vector.tensor_add(out=stage[:, m, :], in0=po,
                                     in1=(b2s_sb if lo else b2_sb))
                sq = sqpool.tile([P, D], f32, tag="sq")
                nc.scalar.activation(
                    out=sq, in_=stage[:, m, :], func=AF.Square,
                    accum_out=q[:, m:m + 1],
                )

            # f = sc / (sqrt(q) + eps); y = stage * f
            eps = 1e-8 * (S_OUT if lo else 1.0)
            qs = smpool.tile([P, 4], f32, tag="qs", name="qs")[:, :m_tiles]
            nc.scalar.activation(out=qs, in_=q, func=AF.Sqrt)
            nc.vector.tensor_scalar_add(out=qs, in0=qs, scalar1=eps)
            nc.vector.reciprocal(out=qs, in_=qs)
            f = smpool.tile([P, 4], f32, tag="f", name="f")[:, :m_tiles]
            nc.vector.tensor_mul(out=f, in0=qs,
                                 in1=sc_sb[:, B // P:B // P + m_tiles])
            for m in range(m_tiles):
                nc.vector.tensor_scalar_mul(
                    out=stage[:, m, :], in0=stage[:, m, :],
                    scalar1=f[:, m:m + 1],
                )
                last = bi == n_blk - 1 and m == m_tiles - 1
                eng = nc.sync if last else nc.gpsimd
                eng.dma_start(out=y_t[:, B // P + m, :], in_=stage[:, m, :])

    nc.compile()
    return nc


def _get_nc():
    if "nc" not in _nc_cache:
        _nc_cache["nc"] = _build_nc()
    return _nc_cache["nc"]


LAST_EXEC_NS = None
LAST_TRACE = None


def _install_axon_ntff_hook():
    """Register antenv.axon_hooks shim driving NTFF capture via the axon .so."""
    import contextlib
    import ctypes
    import types

    if "antenv.axon_hooks" in sys.modules:
        return
    lib = ctypes.CDLL("/opt/axon/libaxon_pjrt.so")
    if not hasattr(lib, "axon_start_nrt_profile"):
        return
    lib.axon_start_nrt_profile.argtypes = [ctypes.POINTER(ctypes.c_int64),
                                           ctypes.c_size_t]
    lib.axon_start_nrt_profile.restype = ctypes.c_int64
    lib.axon_stop_nrt_profile.argtypes = [ctypes.c_char_p]
    lib.axon_stop_nrt_profile.restype = ctypes.c_int64

    @contextlib.contextmanager
    def _hook(output_dir, device_ids):
        import jax
        jax.devices()
        if device_ids:
            ids = (ctypes.c_int64 * len(device_ids))(*device_ids)
            rc = lib.axon_start_nrt_profile(ids, len(device_ids))
        else:
            rc = lib.axon_start_nrt_profile(None, 0)
        if rc != 0:
            raise RuntimeError(f"axon_start_nrt_profile rc={rc}")
        try:
            yield
        finally:
            n = lib.axon_stop_nrt_profile(str(output_dir).encode())
            print(f"ntff capture: {n} file(s) -> {output_dir}", file=sys.stderr)

    mod = types.ModuleType("antenv.axon_hooks")
    mod.get_axon_ntff_profile_hook = lambda: _hook
    sys.modules["antenv.axon_hooks"] = mod
    import antenv
    antenv.axon_hooks = mod


def _gating(x, w_gate, k):
    """Top-k gating computed exactly like the reference (CPU jax, fp32)."""
    import jax
    import jax.numpy as jnp

    cpu = jax.devices("cpu")[0]
    with jax.default_device(cpu):
        xj = jnp.asarray(x)
        logits = xj @ jnp.asarray(w_gate)
        top_vals, top_idx = jax.lax.top_k(logits, k)
        top_gates = jax.nn.softmax(top_vals, axis=-1)
        init_norm = jnp.linalg.norm(xj, axis=-1)
        return (np.asarray(top_idx), np.asarray(top_gates, np.float32),
                np.asarray(init_norm, np.float32))


def kernel(x, w_gate, w1, b1, w2, b2, k):
    from concourse.bass_utils import run_bass_kernel_spmd

    x = np.asarray(x, np.float32)
    w_gate = np.asarray(w_gate, np.float32)
    w1 = np.asarray(w1, np.float32)
    b1 = np.asarray(b1, np.float32)
    w2 = np.asarray(w2, np.float32)
    b2 = np.asarray(b2, np.float32)
    k = int(np.asarray(k))
    n, d = x.shape
    e = w_gate.shape[1]

    top_idx, top_gates, init_norm = _gating(x, w_gate, k)

    idxs, scs = [], []
    for ei in range(e):
        tok, slot = np.nonzero(top_idx == ei)
        w = top_gates[tok, slot] * init_norm[tok]
        order = np.argsort(-w)
        assert len(tok) <= C_TOT, f"expert {ei} load {len(tok)} > {C_TOT}"
        idxs.append(tok[order])
        scs.append(w[order])

    nc = _get_nc()

    in_maps = []
    for ei in range(e):
        tok = idxs[ei]
        n_hi = min(len(tok), C_HI)
        xg_hi = np.zeros((C_HI, d), np.float32)
        xg_hi[:n_hi] = x[tok[:n_hi]]
        xg_lo = np.zeros((C_LO, d), np.float32)
        xg_lo[:len(tok) - n_hi] = x[tok[n_hi:]]
        sce = np.zeros((C_TOT,), np.float32)
        sce[:n_hi] = scs[ei][:n_hi]
        sce[C_HI:C_HI + len(tok) - n_hi] = scs[ei][n_hi:]
        sce = np.ascontiguousarray(sce.reshape(C_TOT // P, P).T)
        in_maps.append({
            "xT": _tile_xT(xg_hi, HI_BLOCKS, BF16),
            "xT8": _tile_xT(_q8(xg_lo, S_X).astype(np.float32), LO_BLOCKS,
                            np.float32).astype(F8),
            "w1": _tile_w1(w1[ei], BF16),
            "w18": _tile_w1(_q8(w1[ei], S_W1).astype(np.float32),
                            np.float32).astype(F8),
            "b1": np.ascontiguousarray(b1[ei].reshape(NH, P).T),
            "b1s": np.ascontiguousarray((S_H * b1[ei]).reshape(NH, P).T),
            "w2": _tile_w2(w2[ei], BF16),
            "w28": _tile_w2(_q8(w2[ei], S_W2).astype(np.float32),
                            np.float32).astype(F8),
            "b2": np.ascontiguousarray(b2[ei]),
            "b2s": np.ascontiguousarray(S_OUT * b2[ei]),
            "sc": sce,
        })

    trace = bool(int(os.environ.get("MOE_TRACE", "0")))
    kwargs = {}
    if trace:
        _install_axon_ntff_hook()
        tdir = os.environ.get("MOE_TRACE_DIR")
        if tdir:
            os.makedirs(tdir, exist_ok=True)
            kwargs["tmpdir"] = tdir
        kwargs["trace_cores"] = [0]
    res = run_bass_kernel_spmd(
        nc, in_maps, core_ids=list(range(e)), trace=trace, **kwargs,
    )
    global LAST_EXEC_NS, LAST_TRACE
    LAST_EXEC_NS = res.exec_time_ns
    LAST_TRACE = res.instructions_and_trace
    if res.exec_time_ns is not None:
        print(f"HW exec time: {res.exec_time_ns} ns", file=sys.stderr)

    y = np.zeros((n, d), np.float32)
    for ei in range(e):
        tok = idxs[ei]
        n_hi = min(len(tok), C_HI)
        ydev = res.results[ei]["y"]
        y[tok[:n_hi]] += ydev[:n_hi]
        y[tok[n_hi:]] += ydev[C_HI:C_HI + len(tok) - n_hi]
    return y


# revision 5
# speedup vs baseline: 1.0815x; 1.0014x over previous
"""Expert-parallel mixed-precision MoE kernel for Trainium2 (8 NeuronCores).

Problem: top-2 MoE, N=8192 tokens, D=1024, H=4096, E=8 experts.
Strategy (expert parallel, per-token mixed precision):
  - Host: compute gating (logits -> top-k -> softmax) exactly as the
    reference does (CPU jax, fp32), dispatch tokens to their experts.
  - Core e holds expert e's tokens, split into two tiers by combine
    weight g*||x||: the top C_HI=1664 run a bf16 pipeline, the rest
    (<=C_LO=512) run an fp8-e4m3 DoubleRow pipeline (2x matmul rate,
    measured 216ns for a 256-deep x 512-wide MM).  Output combine
    rescale:  y = mlp(x) * (gate * ||x||) / ||mlp(x)||.
    fp8 error only hits low-gate tokens; exact rel err ~1.5e-2.
  - Host: scatter-add per-expert outputs back to the [N, D] result.

Device kernel (per core, fp32 PSUM accumulation):
  Token blocks: hi [512, 384, 384, 384] bf16, lo [512] fp8.  Layer 1
  computes hT [H, R] (H on partitions) by streaming w1 per h-tile,
  relu-ing into two token-half hT tiles (3-buffer pool, so block n+1's
  layer 1 never serializes against block n's layer 2). Layer 2
  accumulates out[R, D] in PSUM over h with w2 resident in SBUF (bf16
  64KB + fp8 32KB per partition).  Per-m-tile epilogue straight from
  PSUM (b2 is zero): ACT Square with accum -> q, ACT Rsqrt, DVE mul
  by sc, DVE scale, DMA out — so y DMAs spread through layer 2 and
  the kernel tail only flushes one 512KB transfer.
  fp8 scales (x*16, w1*1024, h*32, w2*1024) fold into the relu
  scale/bias; the final y is exact fp32 (stage*f cancels the 32768x
  product scale; padded rows hit rsqrt(0)=inf but are never read).

  DMA: block-0 xt is loaded in four 2-k-tile chunks interleaved with
  the first h-group's matmuls; w2/w2_f8 ride the sync HWDGE FIFO
  paced between w1 chunks of the first two blocks; y outputs go on
  the gpsimd SWDGE queue except the last m-tile (sync).
"""

import os
import sys

import numpy as np

if "/opt/trn_rl_repo" not in sys.path:
    sys.path.insert(0, "/opt/trn_rl_repo")

import ml_dtypes

N, D, H, E = 8192, 1024, 4096, 8
P = 128
NK = D // P   # 8
NH = H // P   # 32
BF16 = ml_dtypes.bfloat16
F8 = ml_dtypes.float8_e4m3

C_HI, C_LO = 1664, 512
HI_BLOCKS = [512, 384, 384, 384]
LO_BLOCKS = [512]
C_TOT = C_HI + C_LO
S_X, S_W1, S_H, S_W2 = 16.0, 1024.0, 32.0, 1024.0
S_L1 = S_X * S_W1            # layer-1 psum scale
S_OUT = S_H * S_W2           # layer-2 psum scale

assert sum(HI_BLOCKS) == C_HI and sum(LO_BLOCKS) == C_LO

_nc_cache = {}


def _tile_w1(w1e, dt):
    """[D, H] fp32 -> [P, NH, NK, P] with w1t[p,h,k,j] = w1e[k*P+p, h*P+j]."""
    return np.ascontiguousarray(
        w1e.reshape(NK, P, NH, P).transpose(1, 2, 0, 3).astype(dt))


def _tile_w2(w2e, dt):
    """[H, D] fp32 -> [P, NH, D] with w2t[p,h,d] = w2e[h*P+p, d]."""
    return np.ascontiguousarray(
        w2e.reshape(NH, P, D).transpose(1, 0, 2).astype(dt))


def _tile_xT(xg, blocks, dt):
    """[C, D] fp32 (padded) -> [P, NK*C], per-block [k, j] segments."""
    C = xg.shape[0]
    out = np.zeros((P, NK * C), dt)
    B = 0
    for R in blocks:
        seg = xg[B:B + R].T.reshape(NK, P, R).transpose(1, 0, 2)
        out[:, NK * B:NK * (B + R)] = seg.reshape(P, NK * R)
        B += R
    return out


def _q8(a, scale):
    return np.clip(np.asarray(a, np.float32) * scale, -240, 240).astype(F8)


def _build_nc(has_b2):
    from contextlib import ExitStack

    import concourse.bass as bass
    import concourse.mybir as mybir
    import concourse.tile as tile
    from concourse import bacc

    f32 = mybir.dt.float32
    bf16 = mybir.dt.bfloat16
    f8 = mybir.dt.float8e4
    AF = mybir.ActivationFunctionType
    DR = mybir.MatmulPerfMode.DoubleRow

    nc = bacc.Bacc(trn_type="TRN2", num_devices=E)
    xT = nc.dram_tensor("xT", [P, NK * C_HI], bf16, kind="ExternalInput")
    xT8 = nc.dram_tensor("xT8", [P, NK * C_LO], f8, kind="ExternalInput")
    w1 = nc.dram_tensor("w1", [P, NH, NK, P], bf16, kind="ExternalInput")
    w18 = nc.dram_tensor("w18", [P, NH, NK, P], f8, kind="ExternalInput")
    b1 = nc.dram_tensor("b1", [P, NH], f32, kind="ExternalInput")
    b1s = nc.dram_tensor("b1s", [P, NH], f32, kind="ExternalInput")
    w2 = nc.dram_tensor("w2", [P, NH, D], bf16, kind="ExternalInput")
    w28 = nc.dram_tensor("w28", [P, NH, D], f8, kind="ExternalInput")
    b2 = nc.dram_tensor("b2", [D], f32, kind="ExternalInput")
    b2s = nc.dram_tensor("b2s", [D], f32, kind="ExternalInput")
    sc = nc.dram_tensor("sc", [P, C_TOT // P], f32, kind="ExternalInput")
    y = nc.dram_tensor("y", [C_TOT, D], f32, kind="ExternalOutput")

    y_t = y.ap().rearrange("(o p) d -> p o d", p=P)

    with tile.TileContext(nc) as tc, ExitStack() as ctx:
        singles = ctx.enter_context(tc.tile_pool(name="singles", bufs=1))
        xpool = ctx.enter_context(tc.tile_pool(name="xpool", bufs=2))
        w1pool = ctx.enter_context(tc.tile_pool(name="w1pool", bufs=4))
        hpool = ctx.enter_context(tc.tile_pool(name="hpool", bufs=3))
        stpool = ctx.enter_context(tc.tile_pool(name="stpool", bufs=2))
        sqpool = ctx.enter_context(tc.tile_pool(name="sqpool", bufs=1))
        smpool = ctx.enter_context(tc.tile_pool(name="smpool", bufs=4))
        psh = ctx.enter_context(tc.tile_pool(name="psh", bufs=3, space="PSUM"))
        pso = ctx.enter_context(tc.tile_pool(name="pso", bufs=2, space="PSUM"))

        # --- constants (gpsimd SWDGE queue; small) ---
        b1_sb = singles.tile([P, NH], f32)
        nc.gpsimd.dma_start(out=b1_sb, in_=b1.ap())
        b1s_sb = singles.tile([P, NH], f32)
        nc.gpsimd.dma_start(out=b1s_sb, in_=b1s.ap())
        sc_sb = singles.tile([P, C_TOT // P], f32)
        nc.gpsimd.dma_start(out=sc_sb, in_=sc.ap())
        if has_b2:
            b2_sb = singles.tile([P, D], f32)
            b2_bcast = bass.AP(tensor=b2.ap().tensor, offset=b2.ap().offset,
                               ap=[[0, P], *b2.ap().ap])
            nc.gpsimd.dma_start(out=b2_sb, in_=b2_bcast)
            b2s_sb = singles.tile([P, D], f32)
            b2s_bcast = bass.AP(tensor=b2s.ap().tensor, offset=b2s.ap().offset,
                                ap=[[0, P], *b2s.ap().ap])
            nc.gpsimd.dma_start(out=b2s_sb, in_=b2s_bcast)
        # w2 / w2_f8 are paced between w1 chunks on the sync FIFO below.
        w2_sb = singles.tile([P, NH, D], bf16)
        w28_sb = singles.tile([P, NH, D], f8)

        blocks = ([("hi", B, R) for B, R in
                   zip(np.cumsum([0] + HI_BLOCKS[:-1]).tolist(), HI_BLOCKS)]
                  + [("lo", B, R) for B, R in
                     zip((C_HI + np.cumsum([0] + LO_BLOCKS[:-1])).tolist(),
                         LO_BLOCKS)])
        n_blk = len(blocks)

        for bi, (tier, B, R) in enumerate(blocks):
            m_tiles = (R + P - 1) // P
            lo = tier == "lo"
            first = bi == 0
            Ra = min(R, 256)          # token-half split for hi hT tiles

            # --- fetch x block ---
            if lo:
                xt = xpool.tile([P, NK, 512], f8, tag="xt", name="xt8")[:, :, :R]
                nc.sync.dma_start(
                    out=xt,
                    in_=xT8.ap()[:, NK * (B - C_HI):NK * (B - C_HI + R)]
                    .rearrange("p (k j) -> p k j", k=NK))
            else:
                xt = xpool.tile([P, NK, 512], bf16, tag="xt", name="xt")[:, :, :R]
                if not first:
                    nc.sync.dma_start(
                        out=xt,
                        in_=xT.ap()[:, NK * B:NK * (B + R)]
                        .rearrange("p (k j) -> p k j", k=NK))
                # block 0: chunked loads interleaved with the h=0 matmuls

            # --- layer 1: hT[h, tok] (H on partitions) ---
            if lo:
                hts = [hpool.tile([P, NH, 512], f8, tag="hT",
                                  name=f"hT{bi}")[:, :, :R]]
            else:
                hts = [hpool.tile([P, NH, 256], bf16, tag="hT",
                                  name=f"hTa{bi}")[:, :, :Ra],
                       hpool.tile([P, NH, 256], bf16, tag="hT",
                                  name=f"hTb{bi}")[:, :, :R - Ra]]
            for h in range(NH):
                w1c = w1pool.tile([P, NK, P], f8 if lo else bf16,
                                  tag="w1c", name=f"w1c{bi}_{h}")
                nc.sync.dma_start(out=w1c,
                                  in_=(w18 if lo else w1).ap()[:, h])
                if bi == 0 and h % 4 == 3:
                    # w2 rows ride the same FIFO, paced between w1 chunks
                    nc.sync.dma_start(out=w2_sb[:, h - 3:h + 1, :],
                                      in_=w2.ap()[:, h - 3:h + 1, :])
                if bi == 1 and h % 4 == 3:
                    nc.sync.dma_start(out=w28_sb[:, h - 3:h + 1, :],
                                      in_=w28.ap()[:, h - 3:h + 1, :])
                ps = psh.tile([P, 512], f32, tag="ph", name="ph")[:, :R]
                if first and h == 0:
                    for c in range(4):
                        nc.sync.dma_start(
                            out=xt[:, 2 * c:2 * c + 2, :],
                            in_=xT.ap()[:, NK * B + 2 * c * 512:
                                        NK * B + (2 * c + 2) * 512]
                            .rearrange("p (k j) -> p k j", k=2))
                        for k2 in range(2):
                            nc.tensor.matmul(
                                ps, lhsT=w1c[:, 2 * c + k2, :],
                                rhs=xt[:, 2 * c + k2, :],
                                start=(c == 0 and k2 == 0),
                                stop=(c == 3 and k2 == 1))
                elif lo:
                    for kp in range(NK // 2):
                        nc.tensor.matmul(
                            ps, lhsT=w1c[:, 2 * kp:2 * kp + 2, :],
                            rhs=xt[:, 2 * kp:2 * kp + 2, :],
                            start=(kp == 0), stop=(kp == NK // 2 - 1),
                            perf_mode=DR)
                else:
                    for k in range(NK):
                        nc.tensor.matmul(
                            ps, lhsT=w1c[:, k, :], rhs=xt[:, k, :],
                            start=(k == 0), stop=(k == NK - 1))
                if lo:
                    nc.scalar.activation(
                        out=hts[0][:, h, :], in_=ps, func=AF.Relu,
                        bias=b1s_sb[:, h:h + 1], scale=S_H / S_L1)
                else:
                    nc.scalar.activation(
                        out=hts[0][:, h, :], in_=ps[:, :Ra], func=AF.Relu,
                        bias=b1_sb[:, h:h + 1], scale=1.0)
                    nc.scalar.activation(
                        out=hts[1][:, h, :], in_=ps[:, Ra:], func=AF.Relu,
                        bias=b1_sb[:, h:h + 1], scale=1.0)

            # --- layer 2 + per-m-tile epilogue ---
            q = smpool.tile([P, 4], f32, tag="q", name="q")[:, :m_tiles]
            qs = smpool.tile([P, 4], f32, tag="qs", name="qs")[:, :m_tiles]
            f = smpool.tile([P, 4], f32, tag="f", name="f")[:, :m_tiles]
            for m in range(m_tiles):
                po = pso.tile([P, D], f32, tag="po")
                if lo:
                    lhs_src = hts[0][:, :, m * P:(m + 1) * P]
                    for hp in range(NH // 2):
                        for n2 in range(2):
                            nc.tensor.matmul(
                                po[:, n2 * 512:(n2 + 1) * 512],
                                lhsT=lhs_src[:, 2 * hp:2 * hp + 2, :],
                                rhs=w28_sb[:, 2 * hp:2 * hp + 2,
                                           n2 * 512:(n2 + 1) * 512],
                                start=(hp == 0), stop=(hp == NH // 2 - 1),
                                perf_mode=DR)
                else:
                    ht = hts[m // 2]
                    o = (m % 2) * P
                    for h in range(NH):
                        for n2 in range(2):
                            nc.tensor.matmul(
                                po[:, n2 * 512:(n2 + 1) * 512],
                                lhsT=ht[:, h, o:o + P],
                                rhs=w2_sb[:, h, n2 * 512:(n2 + 1) * 512],
                                start=(h == 0), stop=(h == NH - 1))
                stage = stpool.tile([P, D], f32, tag="stage", name="stage")
                if has_b2:
                    nc.vector.tensor_add(out=stage, in0=po,
                                         in1=(b2s_sb if lo else b2_sb))
                    src = stage
                else:
                    src = po
                sq = sqpool.tile([P, D], f32, tag="sq")
                nc.scalar.activation(out=sq, in_=src, func=AF.Square,
                                     accum_out=q[:, m:m + 1])
                nc.scalar.activation(out=qs[:, m:m + 1], in_=q[:, m:m + 1],
                                     func=AF.Sqrt)
                nc.vector.reciprocal(out=qs[:, m:m + 1], in_=qs[:, m:m + 1])
                nc.vector.tensor_mul(out=f[:, m:m + 1], in0=qs[:, m:m + 1],
                                     in1=sc_sb[:, B // P + m:B // P + m + 1])
                nc.vector.tensor_scalar_mul(out=stage, in0=src,
                                            scalar1=f[:, m:m + 1])
                last = bi == n_blk - 1 and m == m_tiles - 1
                eng = nc.sync if last else nc.gpsimd
                eng.dma_start(out=y_t[:, B // P + m, :], in_=stage)

    nc.compile()
    return nc


def _get_nc(has_b2):
    key = ("nc", has_b2)
    if key not in _nc_cache:
        _nc_cache[key] = _build_nc(has_b2)
    return _nc_cache[key]


LAST_EXEC_NS = None
LAST_TRACE = None


def _install_axon_ntff_hook():
    """Register antenv.axon_hooks shim driving NTFF capture via the axon .so."""
    import contextlib
    import ctypes
    import types

    if "antenv.axon_hooks" in sys.modules:
        return
    lib = ctypes.CDLL("/opt/axon/libaxon_pjrt.so")
    if not hasattr(lib, "axon_start_nrt_profile"):
        return
    lib.axon_start_nrt_profile.argtypes = [ctypes.POINTER(ctypes.c_int64),
                                           ctypes.c_size_t]
    lib.axon_start_nrt_profile.restype = ctypes.c_int64
    lib.axon_stop_nrt_profile.argtypes = [ctypes.c_char_p]
    lib.axon_stop_nrt_profile.restype = ctypes.c_int64

    @contextlib.contextmanager
    def _hook(output_dir, device_ids):
        import jax
        jax.devices()
        if device_ids:
            ids = (ctypes.c_int64 * len(device_ids))(*device_ids)
            rc = lib.axon_start_nrt_profile(ids, len(device_ids))
        else:
            rc = lib.axon_start_nrt_profile(None, 0)
        if rc != 0:
            raise RuntimeError(f"axon_start_nrt_profile rc={rc}")
        try:
            yield
        finally:
            n = lib.axon_stop_nrt_profile(str(output_dir).encode())
            print(f"ntff capture: {n} file(s) -> {output_dir}", file=sys.stderr)

    mod = types.ModuleType("antenv.axon_hooks")
    mod.get_axon_ntff_profile_hook = lambda: _hook
    sys.modules["antenv.axon_hooks"] = mod
    import antenv
    antenv.axon_hooks = mod


def _gating(x, w_gate, k):
    """Top-k gating computed exactly like the reference (CPU jax, fp32)."""
    import jax
    import jax.numpy as jnp

    cpu = jax.devices("cpu")[0]
    with jax.default_device(cpu):
        xj = jnp.asarray(x)
        logits = xj @ jnp.asarray(w_gate)
        top_vals, top_idx = jax.lax.top_k(logits, k)
        top_gates = jax.nn.softmax(top_vals, axis=-1)
        init_norm = jnp.linalg.norm(xj, axis=-1)
        return (np.asarray(top_idx), np.asarray(top_gates, np.float32),
                np.asarray(init_norm, np.float32))


def kernel(x, w_gate, w1, b1, w2, b2, k):
    from concourse.bass_utils import run_bass_kernel_spmd

    x = np.asarray(x, np.float32)
    w_gate = np.asarray(w_gate, np.float32)
    w1 = np.asarray(w1, np.float32)
    b1 = np.asarray(b1, np.float32)
    w2 = np.asarray(w2, np.float32)
    b2 = np.asarray(b2, np.float32)
    k = int(np.asarray(k))
    n, d = x.shape
    e = w_gate.shape[1]

    top_idx, top_gates, init_norm = _gating(x, w_gate, k)

    idxs, scs = [], []
    for ei in range(e):
        tok, slot = np.nonzero(top_idx == ei)
        w = top_gates[tok, slot] * init_norm[tok]
        order = np.argsort(-w)
        assert len(tok) <= C_TOT, f"expert {ei} load {len(tok)} > {C_TOT}"
        idxs.append(tok[order])
        scs.append(w[order])

    has_b2 = bool(np.any(b2))
    nc = _get_nc(has_b2)

    in_maps = []
    for ei in range(e):
        tok = idxs[ei]
        n_hi = min(len(tok), C_HI)
        xg_hi = np.zeros((C_HI, d), np.float32)
        xg_hi[:n_hi] = x[tok[:n_hi]]
        xg_lo = np.zeros((C_LO, d), np.float32)
        xg_lo[:len(tok) - n_hi] = x[tok[n_hi:]]
        sce = np.zeros((C_TOT,), np.float32)
        sce[:n_hi] = scs[ei][:n_hi]
        sce[C_HI:C_HI + len(tok) - n_hi] = scs[ei][n_hi:]
        sce = np.ascontiguousarray(sce.reshape(C_TOT // P, P).T)
        in_maps.append({
            "xT": _tile_xT(xg_hi, HI_BLOCKS, BF16),
            "xT8": _tile_xT(_q8(xg_lo, S_X).astype(np.float32), LO_BLOCKS,
                            np.float32).astype(F8),
            "w1": _tile_w1(w1[ei], BF16),
            "w18": _tile_w1(_q8(w1[ei], S_W1).astype(np.float32),
                            np.float32).astype(F8),
            "b1": np.ascontiguousarray(b1[ei].reshape(NH, P).T),
            "b1s": np.ascontiguousarray((S_H * b1[ei]).reshape(NH, P).T),
            "w2": _tile_w2(w2[ei], BF16),
            "w28": _tile_w2(_q8(w2[ei], S_W2).astype(np.float32),
                            np.float32).astype(F8),
            "b2": np.ascontiguousarray(b2[ei]),
            "b2s": np.ascontiguousarray(S_OUT * b2[ei]),
            "sc": sce,
        })

    trace = bool(int(os.environ.get("MOE_TRACE", "0")))
    kwargs = {}
    if trace:
        _install_axon_ntff_hook()
        tdir = os.environ.get("MOE_TRACE_DIR")
        if tdir:
            os.makedirs(tdir, exist_ok=True)
            kwargs["tmpdir"] = tdir
        kwargs["trace_cores"] = [0]
    res = run_bass_kernel_spmd(
        nc, in_maps, core_ids=list(range(e)), trace=trace, **kwargs,
    )
    global LAST_EXEC_NS, LAST_TRACE
    LAST_EXEC_NS = res.exec_time_ns
    LAST_TRACE = res.instructions_and_trace
    if res.exec_time_ns is not None:
        print(f"HW exec time: {res.exec_time_ns} ns", file=sys.stderr)

    y = np.zeros((n, d), np.float32)
    for ei in range(e):
        tok = idxs[ei]
        n_hi = min(len(tok), C_HI)
        ydev = res.results[ei]["y"]
        y[tok[:n_hi]] += ydev[:n_hi]
        y[tok[n_hi:]] += ydev[C_HI:C_HI + len(tok) - n_hi]
    return y


# revision 8
# speedup vs baseline: 1.1314x; 1.0462x over previous
"""Expert-parallel mixed-precision MoE kernel for Trainium2 (8 NeuronCores).

Problem: top-2 MoE, N=8192 tokens, D=1024, H=4096, E=8 experts.
Strategy (expert parallel, per-token mixed precision):
  - Host: compute gating (logits -> top-k -> softmax) exactly as the
    reference does (CPU jax, fp32), dispatch tokens to their experts.
  - Core e holds expert e's tokens, split into two tiers by combine
    weight g*||x||: the top C_HI=1664 run a bf16 pipeline, the rest
    (<=C_LO=512) run an fp8-e4m3 DoubleRow pipeline (2x matmul rate,
    measured 216ns for a 256-deep x 512-wide MM).  Output combine
    rescale:  y = mlp(x) * (gate * ||x||) / ||mlp(x)||.
    fp8 error only hits low-gate tokens; exact rel err ~1.5e-2.
  - Host: scatter-add per-expert outputs back to the [N, D] result.

Device kernel (per core, fp32 PSUM accumulation):
  Token blocks: hi [512, 384, 384, 384] bf16, lo [512] fp8.  Layer 1
  computes hT [H, R] (H on partitions) by streaming w1 per h-tile,
  relu-ing into two token-half hT tiles (3-buffer pool, so block n+1's
  layer 1 never serializes against block n's layer 2). Layer 2
  accumulates out[R, D] in PSUM over h with w2 resident in SBUF (bf16
  64KB + fp8 32KB per partition).  Per-m-tile epilogue straight from
  PSUM (b2 is zero): ACT Square with accum -> q, ACT Rsqrt, DVE mul
  by sc, DVE scale, DMA out — so y DMAs spread through layer 2 and
  the kernel tail only flushes one 512KB transfer.
  fp8 scales (x*16, w1*1024, h*32, w2*1024) fold into the relu
  scale/bias; the final y is exact fp32 (stage*f cancels the 32768x
  product scale; padded rows hit rsqrt(0)=inf but are never read).

  DMA: block-0 xt is loaded in four 2-k-tile chunks interleaved with
  the first h-group's matmuls; w2/w2_f8 ride the sync HWDGE FIFO
  paced between w1 chunks of the first two blocks; y outputs go on
  the gpsimd SWDGE queue except the last m-tile (sync).
"""

import os
import sys

import numpy as np

if "/opt/trn_rl_repo" not in sys.path:
    sys.path.insert(0, "/opt/trn_rl_repo")

import ml_dtypes

N, D, H, E = 8192, 1024, 4096, 8
P = 128
NK = D // P   # 8
NH = H // P   # 32
BF16 = ml_dtypes.bfloat16
F8 = ml_dtypes.float8_e4m3

C_HI, C_LO = 1664, 512
HI_BLOCKS = [512, 384, 384, 384]
LO_BLOCKS = [512]
C_TOT = C_HI + C_LO
S_X, S_W1, S_H, S_W2 = 16.0, 1024.0, 32.0, 1024.0
S_L1 = S_X * S_W1            # layer-1 psum scale
S_OUT = S_H * S_W2           # layer-2 psum scale

assert sum(HI_BLOCKS) == C_HI and sum(LO_BLOCKS) == C_LO

_nc_cache = {}


def _tile_w1(w1e, dt):
    """[D, H] fp32 -> [P, NH, NK, P] with w1t[p,h,k,j] = w1e[k*P+p, h*P+j]."""
    return np.ascontiguousarray(
        w1e.reshape(NK, P, NH, P).transpose(1, 2, 0, 3).astype(dt))


def _tile_w2(w2e, dt):
    """[H, D] fp32 -> [P, NH, D] with w2t[p,h,d] = w2e[h*P+p, d]."""
    return np.ascontiguousarray(
        w2e.reshape(NH, P, D).transpose(1, 0, 2).astype(dt))


def _tile_xT(xg, blocks, dt):
    """[C, D] fp32 (padded) -> [P, NK*C], per-block [k, j] segments."""
    C = xg.shape[0]
    out = np.zeros((P, NK * C), dt)
    B = 0
    for R in blocks:
        seg = xg[B:B + R].T.reshape(NK, P, R).transpose(1, 0, 2)
        out[:, NK * B:NK * (B + R)] = seg.reshape(P, NK * R)
        B += R
    return out


def _q8(a, scale):
    return np.clip(np.asarray(a, np.float32) * scale, -240, 240).astype(F8)


def _build_nc(has_b2):
    from contextlib import ExitStack

    import concourse.bass as bass
    import concourse.mybir as mybir
    import concourse.tile as tile
    from concourse import bacc

    f32 = mybir.dt.float32
    bf16 = mybir.dt.bfloat16
    f8 = mybir.dt.float8e4
    AF = mybir.ActivationFunctionType
    DR = mybir.MatmulPerfMode.DoubleRow

    nc = bacc.Bacc(trn_type="TRN2", num_devices=E)
    xT = nc.dram_tensor("xT", [P, NK * C_HI], bf16, kind="ExternalInput")
    xT8 = nc.dram_tensor("xT8", [P, NK * C_LO], f8, kind="ExternalInput")
    w1 = nc.dram_tensor("w1", [P, NH, NK, P], bf16, kind="ExternalInput")
    w18 = nc.dram_tensor("w18", [P, NH, NK, P], f8, kind="ExternalInput")
    b1 = nc.dram_tensor("b1", [P, NH], f32, kind="ExternalInput")
    b1s = nc.dram_tensor("b1s", [P, NH], f32, kind="ExternalInput")
    w2 = nc.dram_tensor("w2", [P, NH, D], bf16, kind="ExternalInput")
    w28 = nc.dram_tensor("w28", [P, NH, D], f8, kind="ExternalInput")
    b2 = nc.dram_tensor("b2", [D], f32, kind="ExternalInput")
    b2s = nc.dram_tensor("b2s", [D], f32, kind="ExternalInput")
    sc = nc.dram_tensor("sc", [P, C_TOT // P], f32, kind="ExternalInput")
    y = nc.dram_tensor("y", [C_TOT, D], f32, kind="ExternalOutput")

    y_t = y.ap().rearrange("(o p) d -> p o d", p=P)

    with tile.TileContext(nc) as tc, ExitStack() as ctx:
        singles = ctx.enter_context(tc.tile_pool(name="singles", bufs=1))
        xpool = ctx.enter_context(tc.tile_pool(name="xpool", bufs=2))
        w1pool = ctx.enter_context(tc.tile_pool(name="w1pool", bufs=6))
        hpool = ctx.enter_context(tc.tile_pool(name="hpool", bufs=3))
        stpool = ctx.enter_context(tc.tile_pool(name="stpool", bufs=2))
        sqpool = ctx.enter_context(tc.tile_pool(name="sqpool", bufs=1))
        smpool = ctx.enter_context(tc.tile_pool(name="smpool", bufs=4))
        psh = ctx.enter_context(tc.tile_pool(name="psh", bufs=3, space="PSUM"))
        pso = ctx.enter_context(tc.tile_pool(name="pso", bufs=2, space="PSUM"))

        # --- constants (gpsimd SWDGE queue; small) ---
        b1_sb = singles.tile([P, NH], f32)
        nc.gpsimd.dma_start(out=b1_sb, in_=b1.ap())
        b1s_sb = singles.tile([P, NH], f32)
        nc.gpsimd.dma_start(out=b1s_sb, in_=b1s.ap())
        sc_sb = singles.tile([P, C_TOT // P], f32)
        nc.gpsimd.dma_start(out=sc_sb, in_=sc.ap())
        if has_b2:
            b2_sb = singles.tile([P, D], f32)
            b2_bcast = bass.AP(tensor=b2.ap().tensor, offset=b2.ap().offset,
                               ap=[[0, P], *b2.ap().ap])
            nc.gpsimd.dma_start(out=b2_sb, in_=b2_bcast)
            b2s_sb = singles.tile([P, D], f32)
            b2s_bcast = bass.AP(tensor=b2s.ap().tensor, offset=b2s.ap().offset,
                                ap=[[0, P], *b2s.ap().ap])
            nc.gpsimd.dma_start(out=b2s_sb, in_=b2s_bcast)
        # w2 / w2_f8 are paced between w1 chunks on the sync FIFO below.
        w2_sb = singles.tile([P, NH, D], bf16)
        w28_sb = singles.tile([P, NH, D], f8)

        blocks = ([("hi", B, R) for B, R in
                   zip(np.cumsum([0] + HI_BLOCKS[:-1]).tolist(), HI_BLOCKS)]
                  + [("lo", B, R) for B, R in
                     zip((C_HI + np.cumsum([0] + LO_BLOCKS[:-1])).tolist(),
                         LO_BLOCKS)])
        n_blk = len(blocks)

        def make_xt(bj, eng):
            """Allocate block bj's x tile; DMA it on `eng` (bulk prefetch)."""
            tier_j, B_j, R_j = blocks[bj]
            if tier_j == "lo":
                t = xpool.tile([P, NK, 512], f8, tag="xt", name="xt8")[:, :, :R_j]
                eng.dma_start(
                    out=t,
                    in_=xT8.ap()[:, NK * (B_j - C_HI):NK * (B_j - C_HI + R_j)]
                    .rearrange("p (k j) -> p k j", k=NK))
            else:
                t = xpool.tile([P, NK, 512], bf16, tag="xt", name="xt")[:, :, :R_j]
                eng.dma_start(
                    out=t,
                    in_=xT.ap()[:, NK * B_j:NK * (B_j + R_j)]
                    .rearrange("p (k j) -> p k j", k=NK))
            return t

        xts = {}
        for bi, (tier, B, R) in enumerate(blocks):
            m_tiles = (R + P - 1) // P
            lo = tier == "lo"
            first = bi == 0
            Ra = min(R, 256)          # token-half split for hi hT tiles

            if first:
                # block 0: chunked loads interleaved with the h=0 matmuls
                xt = xpool.tile([P, NK, 512], bf16, tag="xt", name="xt")[:, :, :R]
            else:
                xt = xts.pop(bi)

            # --- layer 1: hT[h, tok] (H on partitions) ---
            if lo:
                hts = [hpool.tile([P, NH, 512], f8, tag="hT",
                                  name=f"hT{bi}")[:, :, :R]]
            else:
                hts = [hpool.tile([P, NH, 256], bf16, tag="hT",
                                  name=f"hTa{bi}")[:, :, :Ra],
                       hpool.tile([P, NH, 256], bf16, tag="hT",
                                  name=f"hTb{bi}")[:, :, :R - Ra]]
            for h in range(NH):
                w1c = w1pool.tile([P, NK, P], f8 if lo else bf16,
                                  tag="w1c", name=f"w1c{bi}_{h}")
                nc.sync.dma_start(out=w1c,
                                  in_=(w18 if lo else w1).ap()[:, h])
                # Bulk prefetch rides the scalar-engine HWDGE queue: the
                # SDMA engines round-robin queues at packet granularity,
                # so these big streams don't starve the w1 JIT chunks.
                if h == 2 and bi + 1 < n_blk:
                    xts[bi + 1] = make_xt(bi + 1, nc.scalar)
                if bi == 0 and h % 8 == 4:
                    nc.scalar.dma_start(out=w2_sb[:, h - 4:h + 4, :],
                                        in_=w2.ap()[:, h - 4:h + 4, :])
                if bi == 1 and h in (4, 20):
                    nc.scalar.dma_start(out=w28_sb[:, h - 4:h + 12, :],
                                        in_=w28.ap()[:, h - 4:h + 12, :])
                ps = psh.tile([P, 512], f32, tag="ph", name="ph")[:, :R]
                if first and h == 0:
                    for c in range(4):
                        nc.sync.dma_start(
                            out=xt[:, 2 * c:2 * c + 2, :],
                            in_=xT.ap()[:, NK * B + 2 * c * 512:
                                        NK * B + (2 * c + 2) * 512]
                            .rearrange("p (k j) -> p k j", k=2))
                        for k2 in range(2):
                            nc.tensor.matmul(
                                ps, lhsT=w1c[:, 2 * c + k2, :],
                                rhs=xt[:, 2 * c + k2, :],
                                start=(c == 0 and k2 == 0),
                                stop=(c == 3 and k2 == 1))
                elif lo:
                    for kp in range(NK // 2):
                        nc.tensor.matmul(
                            ps, lhsT=w1c[:, 2 * kp:2 * kp + 2, :],
                            rhs=xt[:, 2 * kp:2 * kp + 2, :],
                            start=(kp == 0), stop=(kp == NK // 2 - 1),
                            perf_mode=DR)
                else:
                    for k in range(NK):
                        nc.tensor.matmul(
                            ps, lhsT=w1c[:, k, :], rhs=xt[:, k, :],
                            start=(k == 0), stop=(k == NK - 1))
                if lo:
                    nc.scalar.activation(
                        out=hts[0][:, h, :], in_=ps, func=AF.Relu,
                        bias=b1s_sb[:, h:h + 1], scale=S_H / S_L1)
                else:
                    nc.scalar.activation(
                        out=hts[0][:, h, :], in_=ps[:, :Ra], func=AF.Relu,
                        bias=b1_sb[:, h:h + 1], scale=1.0)
                    nc.scalar.activation(
                        out=hts[1][:, h, :], in_=ps[:, Ra:], func=AF.Relu,
                        bias=b1_sb[:, h:h + 1], scale=1.0)

            # --- layer 2 + per-m-tile epilogue ---
            q = smpool.tile([P, 4], f32, tag="q", name="q")[:, :m_tiles]
            qs = smpool.tile([P, 4], f32, tag="qs", name="qs")[:, :m_tiles]
            f = smpool.tile([P, 4], f32, tag="f", name="f")[:, :m_tiles]
            for m in range(m_tiles):
                po = pso.tile([P, D], f32, tag="po")
                if lo:
                    lhs_src = hts[0][:, :, m * P:(m + 1) * P]
                    for hp in range(NH // 2):
                        for n2 in range(2):
                            nc.tensor.matmul(
                                po[:, n2 * 512:(n2 + 1) * 512],
                                lhsT=lhs_src[:, 2 * hp:2 * hp + 2, :],
                                rhs=w28_sb[:, 2 * hp:2 * hp + 2,
                                           n2 * 512:(n2 + 1) * 512],
                                start=(hp == 0), stop=(hp == NH // 2 - 1),
                                perf_mode=DR)
                else:
                    ht = hts[m // 2]
                    o = (m % 2) * P
                    for h in range(NH):
                        for n2 in range(2):
                            nc.tensor.matmul(
                                po[:, n2 * 512:(n2 + 1) * 512],
                                lhsT=ht[:, h, o:o + P],
                                rhs=w2_sb[:, h, n2 * 512:(n2 + 1) * 512],
                                start=(h == 0), stop=(h == NH - 1))
                stage = stpool.tile([P, D], f32, tag="stage", name="stage")
                if has_b2:
                    nc.vector.tensor_add(out=stage, in0=po,
                                         in1=(b2s_sb if lo else b2_sb))
                    src = stage
                else:
                    src = po
                sq = sqpool.tile([P, D], f32, tag="sq")
                nc.scalar.activation(out=sq, in_=src, func=AF.Square,
                                     accum_out=q[:, m:m + 1])
                nc.scalar.activation(out=qs[:, m:m + 1], in_=q[:, m:m + 1],
                                     func=AF.Sqrt)
                nc.vector.reciprocal(out=qs[:, m:m + 1], in_=qs[:, m:m + 1])
                nc.vector.tensor_mul(out=f[:, m:m + 1], in0=qs[:, m:m + 1],
                                     in1=sc_sb[:, B // P + m:B // P + m + 1])
                nc.vector.tensor_scalar_mul(out=stage, in0=src,
                                            scalar1=f[:, m:m + 1])
                last = bi == n_blk - 1 and m == m_tiles - 1
                eng = nc.sync if last else nc.gpsimd
                eng.dma_start(out=y_t[:, B // P + m, :], in_=stage)

    nc.compile()
    return nc


def _get_nc(has_b2):
    key = ("nc", has_b2)
    if key not in _nc_cache:
        _nc_cache[key] = _build_nc(has_b2)
    return _nc_cache[key]


LAST_EXEC_NS = None
LAST_TRACE = None


def _install_axon_ntff_hook():
    """Register antenv.axon_hooks shim driving NTFF capture via the axon .so."""
    import contextlib
    import ctypes
    import types

    if "antenv.axon_hooks" in sys.modules:
        return
    lib = ctypes.CDLL("/opt/axon/libaxon_pjrt.so")
    if not hasattr(lib, "axon_start_nrt_profile"):
        return
    lib.axon_start_nrt_profile.argtypes = [ctypes.POINTER(ctypes.c_int64),
                                           ctypes.c_size_t]
    lib.axon_start_nrt_profile.restype = ctypes.c_int64
    lib.axon_stop_nrt_profile.argtypes = [ctypes.c_char_p]
    lib.axon_stop_nrt_profile.restype = ctypes.c_int64

    @contextlib.contextmanager
    def _hook(output_dir, device_ids):
        import jax
        jax.devices()
        if device_ids:
            ids = (ctypes.c_int64 * len(device_ids))(*device_ids)
            rc = lib.axon_start_nrt_profile(ids, len(device_ids))
        else:
            rc = lib.axon_start_nrt_profile(None, 0)
        if rc != 0:
            raise RuntimeError(f"axon_start_nrt_profile rc={rc}")
        try:
            yield
        finally:
            n = lib.axon_stop_nrt_profile(str(output_dir).encode())
            print(f"ntff capture: {n} file(s) -> {output_dir}", file=sys.stderr)

    mod = types.ModuleType("antenv.axon_hooks")
    mod.get_axon_ntff_profile_hook = lambda: _hook
    sys.modules["antenv.axon_hooks"] = mod
    import antenv
    antenv.axon_hooks = mod


def _gating(x, w_gate, k):
    """Top-k gating computed exactly like the reference (CPU jax, fp32)."""
    import jax
    import jax.numpy as jnp

    cpu = jax.devices("cpu")[0]
    with jax.default_device(cpu):
        xj = jnp.asarray(x)
        logits = xj @ jnp.asarray(w_gate)
        top_vals, top_idx = jax.lax.top_k(logits, k)
        top_gates = jax.nn.softmax(top_vals, axis=-1)
        init_norm = jnp.linalg.norm(xj, axis=-1)
        return (np.asarray(top_idx), np.asarray(top_gates, np.float32),
                np.asarray(init_norm, np.float32))


def kernel(x, w_gate, w1, b1, w2, b2, k):
    from concourse.bass_utils import run_bass_kernel_spmd

    x = np.asarray(x, np.float32)
    w_gate = np.asarray(w_gate, np.float32)
    w1 = np.asarray(w1, np.float32)
    b1 = np.asarray(b1, np.float32)
    w2 = np.asarray(w2, np.float32)
    b2 = np.asarray(b2, np.float32)
    k = int(np.asarray(k))
    n, d = x.shape
    e = w_gate.shape[1]

    top_idx, top_gates, init_norm = _gating(x, w_gate, k)

    idxs, scs = [], []
    for ei in range(e):
        tok, slot = np.nonzero(top_idx == ei)
        w = top_gates[tok, slot] * init_norm[tok]
        order = np.argsort(-w)
        assert len(tok) <= C_TOT, f"expert {ei} load {len(tok)} > {C_TOT}"
        idxs.append(tok[order])
        scs.append(w[order])

    has_b2 = bool(np.any(b2))
    nc = _get_nc(has_b2)

    in_maps = []
    for ei in range(e):
        tok = idxs[ei]
        n_hi = min(len(tok), C_HI)
        xg_hi = np.zeros((C_HI, d), np.float32)
        xg_hi[:n_hi] = x[tok[:n_hi]]
        xg_lo = np.zeros((C_LO, d), np.float32)
        xg_lo[:len(tok) - n_hi] = x[tok[n_hi:]]
        sce = np.zeros((C_TOT,), np.float32)
        sce[:n_hi] = scs[ei][:n_hi]
        sce[C_HI:C_HI + len(tok) - n_hi] = scs[ei][n_hi:]
        sce = np.ascontiguousarray(sce.reshape(C_TOT // P, P).T)
        in_maps.append({
            "xT": _tile_xT(xg_hi, HI_BLOCKS, BF16),
            "xT8": _tile_xT(_q8(xg_lo, S_X).astype(np.float32), LO_BLOCKS,
                            np.float32).astype(F8),
            "w1": _tile_w1(w1[ei], BF16),
            "w18": _tile_w1(_q8(w1[ei], S_W1).astype(np.float32),
                            np.float32).astype(F8),
            "b1": np.ascontiguousarray(b1[ei].reshape(NH, P).T),
            "b1s": np.ascontiguousarray((S_H * b1[ei]).reshape(NH, P).T),
            "w2": _tile_w2(w2[ei], BF16),
            "w28": _tile_w2(_q8(w2[ei], S_W2).astype(np.float32),
                            np.float32).astype(F8),
            "b2": np.ascontiguousarray(b2[ei]),
            "b2s": np.ascontiguousarray(S_OUT * b2[ei]),
            "sc": sce,
        })

    trace = bool(int(os.environ.get("MOE_TRACE", "0")))
    kwargs = {}
    if trace:
        _install_axon_ntff_hook()
        tdir = os.environ.get("MOE_TRACE_DIR")
        if tdir:
            os.makedirs(tdir, exist_ok=True)
            kwargs["tmpdir"] = tdir
        kwargs["trace_cores"] = [0]
    res = run_bass_kernel_spmd(
        nc, in_maps, core_ids=list(range(e)), trace=trace, **kwargs,
    )
    global LAST_EXEC_NS, LAST_TRACE
    LAST_EXEC_NS = res.exec_time_ns
    LAST_TRACE = res.instructions_and_trace
    if res.exec_time_ns is not None:
        print(f"HW exec time: {res.exec_time_ns} ns", file=sys.stderr)

    y = np.zeros((n, d), np.float32)
    for ei in range(e):
        tok = idxs[ei]
        n_hi = min(len(tok), C_HI)
        ydev = res.results[ei]["y"]
        y[tok[:n_hi]] += ydev[:n_hi]
        y[tok[n_hi:]] += ydev[C_HI:C_HI + len(tok) - n_hi]
    return y


# revision 9
# speedup vs baseline: 1.1477x; 1.0144x over previous
"""Expert-parallel mixed-precision MoE kernel for Trainium2 (8 NeuronCores).

Problem: top-2 MoE, N=8192 tokens, D=1024, H=4096, E=8 experts.
Strategy (expert parallel, per-token mixed precision):
  - Host: compute gating (logits -> top-k -> softmax) exactly as the
    reference does (CPU jax, fp32), dispatch tokens to their experts.
  - Core e holds expert e's tokens, split into two tiers by combine
    weight g*||x||: the top C_HI=1664 run a bf16 pipeline, the rest
    (<=C_LO=512) run an fp8-e4m3 DoubleRow pipeline (2x matmul rate,
    measured 216ns for a 256-deep x 512-wide MM).  Output combine
    rescale:  y = mlp(x) * (gate * ||x||) / ||mlp(x)||.
    fp8 error only hits low-gate tokens; exact rel err ~1.5e-2.
  - Host: scatter-add per-expert outputs back to the [N, D] result.

Device kernel (per core, fp32 PSUM accumulation):
  Token blocks: hi [512, 384, 384, 384] bf16, lo [512] fp8.  Layer 1
  computes hT [H, R] (H on partitions) by streaming w1 per h-tile,
  relu-ing into two token-half hT tiles (3-buffer pool, so block n+1's
  layer 1 never serializes against block n's layer 2). Layer 2
  accumulates out[R, D] in PSUM over h with w2 resident in SBUF (bf16
  64KB + fp8 32KB per partition).  Per-m-tile epilogue straight from
  PSUM (b2 is zero): ACT Square with accum -> q, ACT Rsqrt, DVE mul
  by sc, DVE scale, DMA out — so y DMAs spread through layer 2 and
  the kernel tail only flushes one 512KB transfer.
  fp8 scales (x*16, w1*1024, h*32, w2*1024) fold into the relu
  scale/bias; the final y is exact fp32 (stage*f cancels the 32768x
  product scale; padded rows hit rsqrt(0)=inf but are never read).

  DMA: block-0 xt is loaded in four 2-k-tile chunks interleaved with
  the first h-group's matmuls; w2/w2_f8 ride the sync HWDGE FIFO
  paced between w1 chunks of the first two blocks; y outputs go on
  the gpsimd SWDGE queue except the last m-tile (sync).
"""

import os
import sys

import numpy as np

if "/opt/trn_rl_repo" not in sys.path:
    sys.path.insert(0, "/opt/trn_rl_repo")

import ml_dtypes

N, D, H, E = 8192, 1024, 4096, 8
P = 128
NK = D // P   # 8
NH = H // P   # 32
BF16 = ml_dtypes.bfloat16
F8 = ml_dtypes.float8_e4m3

C_HI, C_LO = 1664, 512
HI_BLOCKS = [512, 384, 384, 384]
LO_BLOCKS = [512]
C_TOT = C_HI + C_LO
S_X, S_W1, S_H, S_W2 = 16.0, 1024.0, 32.0, 1024.0
S_L1 = S_X * S_W1            # layer-1 psum scale
S_OUT = S_H * S_W2           # layer-2 psum scale

assert sum(HI_BLOCKS) == C_HI and sum(LO_BLOCKS) == C_LO

_nc_cache = {}


def _tile_w1(w1e, dt):
    """[D, H] fp32 -> [P, NH, NK, P] with w1t[p,h,k,j] = w1e[k*P+p, h*P+j]."""
    return np.ascontiguousarray(
        w1e.reshape(NK, P, NH, P).transpose(1, 2, 0, 3).astype(dt))


def _tile_w2(w2e, dt):
    """[H, D] fp32 -> [P, NH, D] with w2t[p,h,d] = w2e[h*P+p, d]."""
    return np.ascontiguousarray(
        w2e.reshape(NH, P, D).transpose(1, 0, 2).astype(dt))


def _tile_xT(xg, blocks, dt):
    """[C, D] fp32 (padded) -> [P, NK*C], per-block [k, j] segments."""
    C = xg.shape[0]
    out = np.zeros((P, NK * C), dt)
    B = 0
    for R in blocks:
        seg = xg[B:B + R].T.reshape(NK, P, R).transpose(1, 0, 2)
        out[:, NK * B:NK * (B + R)] = seg.reshape(P, NK * R)
        B += R
    return out


def _q8(a, scale):
    return np.clip(np.asarray(a, np.float32) * scale, -240, 240).astype(F8)


def _build_nc(has_b2):
    from contextlib import ExitStack

    import concourse.bass as bass
    import concourse.mybir as mybir
    import concourse.tile as tile
    from concourse import bacc

    f32 = mybir.dt.float32
    bf16 = mybir.dt.bfloat16
    f8 = mybir.dt.float8e4
    AF = mybir.ActivationFunctionType
    DR = mybir.MatmulPerfMode.DoubleRow

    nc = bacc.Bacc(trn_type="TRN2", num_devices=E)
    xT = nc.dram_tensor("xT", [P, NK * C_HI], bf16, kind="ExternalInput")
    xT8 = nc.dram_tensor("xT8", [P, NK * C_LO], f8, kind="ExternalInput")
    w1 = nc.dram_tensor("w1", [P, NH, NK, P], bf16, kind="ExternalInput")
    w18 = nc.dram_tensor("w18", [P, NH, NK, P], f8, kind="ExternalInput")
    b1 = nc.dram_tensor("b1", [P, NH], f32, kind="ExternalInput")
    b1s = nc.dram_tensor("b1s", [P, NH], f32, kind="ExternalInput")
    w2 = nc.dram_tensor("w2", [P, NH, D], bf16, kind="ExternalInput")
    w28 = nc.dram_tensor("w28", [P, NH, D], f8, kind="ExternalInput")
    b2 = nc.dram_tensor("b2", [D], f32, kind="ExternalInput")
    b2s = nc.dram_tensor("b2s", [D], f32, kind="ExternalInput")
    sc = nc.dram_tensor("sc", [P, C_TOT // P], f32, kind="ExternalInput")
    y = nc.dram_tensor("y", [C_TOT, D], f32, kind="ExternalOutput")

    y_t = y.ap().rearrange("(o p) d -> p o d", p=P)

    with tile.TileContext(nc) as tc, ExitStack() as ctx:
        singles = ctx.enter_context(tc.tile_pool(name="singles", bufs=1))
        xpool = ctx.enter_context(tc.tile_pool(name="xpool", bufs=2))
        w1pool = ctx.enter_context(tc.tile_pool(name="w1pool", bufs=6))
        hpool = ctx.enter_context(tc.tile_pool(name="hpool", bufs=3))
        stpool = ctx.enter_context(tc.tile_pool(name="stpool", bufs=2))
        sqpool = ctx.enter_context(tc.tile_pool(name="sqpool", bufs=1))
        smpool = ctx.enter_context(tc.tile_pool(name="smpool", bufs=4))
        psh = ctx.enter_context(tc.tile_pool(name="psh", bufs=2, space="PSUM"))
        pso = ctx.enter_context(tc.tile_pool(name="pso", bufs=3, space="PSUM"))

        # --- constants (gpsimd SWDGE queue; small) ---
        b1_sb = singles.tile([P, NH], f32)
        nc.gpsimd.dma_start(out=b1_sb, in_=b1.ap())
        b1s_sb = singles.tile([P, NH], f32)
        nc.gpsimd.dma_start(out=b1s_sb, in_=b1s.ap())
        sc_sb = singles.tile([P, C_TOT // P], f32)
        nc.gpsimd.dma_start(out=sc_sb, in_=sc.ap())
        if has_b2:
            b2_sb = singles.tile([P, D], f32)
            b2_bcast = bass.AP(tensor=b2.ap().tensor, offset=b2.ap().offset,
                               ap=[[0, P], *b2.ap().ap])
            nc.gpsimd.dma_start(out=b2_sb, in_=b2_bcast)
            b2s_sb = singles.tile([P, D], f32)
            b2s_bcast = bass.AP(tensor=b2s.ap().tensor, offset=b2s.ap().offset,
                                ap=[[0, P], *b2s.ap().ap])
            nc.gpsimd.dma_start(out=b2s_sb, in_=b2s_bcast)
        # w2 / w2_f8 are paced between w1 chunks on the sync FIFO below.
        w2_sb = singles.tile([P, NH, D], bf16)
        w28_sb = singles.tile([P, NH, D], f8)

        blocks = ([("hi", B, R) for B, R in
                   zip(np.cumsum([0] + HI_BLOCKS[:-1]).tolist(), HI_BLOCKS)]
                  + [("lo", B, R) for B, R in
                     zip((C_HI + np.cumsum([0] + LO_BLOCKS[:-1])).tolist(),
                         LO_BLOCKS)])
        n_blk = len(blocks)

        def make_xt(bj, eng):
            """Allocate block bj's x tile; DMA it on `eng` (bulk prefetch)."""
            tier_j, B_j, R_j = blocks[bj]
            if tier_j == "lo":
                t = xpool.tile([P, NK, 512], f8, tag="xt", name="xt8")[:, :, :R_j]
                eng.dma_start(
                    out=t,
                    in_=xT8.ap()[:, NK * (B_j - C_HI):NK * (B_j - C_HI + R_j)]
                    .rearrange("p (k j) -> p k j", k=NK))
            else:
                t = xpool.tile([P, NK, 512], bf16, tag="xt", name="xt")[:, :, :R_j]
                eng.dma_start(
                    out=t,
                    in_=xT.ap()[:, NK * B_j:NK * (B_j + R_j)]
                    .rearrange("p (k j) -> p k j", k=NK))
            return t

        xts = {}
        for bi, (tier, B, R) in enumerate(blocks):
            m_tiles = (R + P - 1) // P
            lo = tier == "lo"
            first = bi == 0
            Ra = min(R, 256)          # token-half split for hi hT tiles

            if first:
                # block 0: chunked loads interleaved with the h=0 matmuls
                xt = xpool.tile([P, NK, 512], bf16, tag="xt", name="xt")[:, :, :R]
            else:
                xt = xts.pop(bi)

            # --- layer 1: hT[h, tok] (H on partitions) ---
            if lo:
                hts = [hpool.tile([P, NH, 512], f8, tag="hT",
                                  name=f"hT{bi}")[:, :, :R]]
            else:
                hts = [hpool.tile([P, NH, 256], bf16, tag="hT",
                                  name=f"hTa{bi}")[:, :, :Ra],
                       hpool.tile([P, NH, 256], bf16, tag="hT",
                                  name=f"hTb{bi}")[:, :, :R - Ra]]
            for h in range(NH):
                w1c = w1pool.tile([P, NK, P], f8 if lo else bf16,
                                  tag="w1c", name=f"w1c{bi}_{h}")
                nc.sync.dma_start(out=w1c,
                                  in_=(w18 if lo else w1).ap()[:, h])
                # Bulk prefetch rides the scalar-engine HWDGE queue: the
                # SDMA engines round-robin queues at packet granularity,
                # so these big streams don't starve the w1 JIT chunks.
                if h == 2 and bi + 1 < n_blk:
                    xts[bi + 1] = make_xt(bi + 1, nc.scalar)
                if bi == 0 and h % 4 == 2:
                    nc.scalar.dma_start(out=w2_sb[:, h - 2:h + 2, :],
                                        in_=w2.ap()[:, h - 2:h + 2, :])
                if bi == 1 and h % 8 == 2:
                    nc.scalar.dma_start(out=w28_sb[:, h - 2:h + 6, :],
                                        in_=w28.ap()[:, h - 2:h + 6, :])
                ps = psh.tile([P, 512], f32, tag="ph", name="ph")[:, :R]
                if first and h == 0:
                    for c in range(4):
                        nc.sync.dma_start(
                            out=xt[:, 2 * c:2 * c + 2, :],
                            in_=xT.ap()[:, NK * B + 2 * c * 512:
                                        NK * B + (2 * c + 2) * 512]
                            .rearrange("p (k j) -> p k j", k=2))
                        for k2 in range(2):
                            nc.tensor.matmul(
                                ps, lhsT=w1c[:, 2 * c + k2, :],
                                rhs=xt[:, 2 * c + k2, :],
                                start=(c == 0 and k2 == 0),
                                stop=(c == 3 and k2 == 1))
                elif lo:
                    for kp in range(NK // 2):
                        nc.tensor.matmul(
                            ps, lhsT=w1c[:, 2 * kp:2 * kp + 2, :],
                            rhs=xt[:, 2 * kp:2 * kp + 2, :],
                            start=(kp == 0), stop=(kp == NK // 2 - 1),
                            perf_mode=DR)
                else:
                    for k in range(NK):
                        nc.tensor.matmul(
                            ps, lhsT=w1c[:, k, :], rhs=xt[:, k, :],
                            start=(k == 0), stop=(k == NK - 1))
                if lo:
                    nc.scalar.activation(
                        out=hts[0][:, h, :], in_=ps, func=AF.Relu,
                        bias=b1s_sb[:, h:h + 1], scale=S_H / S_L1)
                else:
                    nc.scalar.activation(
                        out=hts[0][:, h, :], in_=ps[:, :Ra], func=AF.Relu,
                        bias=b1_sb[:, h:h + 1], scale=1.0)
                    nc.scalar.activation(
                        out=hts[1][:, h, :], in_=ps[:, Ra:], func=AF.Relu,
                        bias=b1_sb[:, h:h + 1], scale=1.0)

            # --- layer 2 + per-m-tile epilogue ---
            q = smpool.tile([P, 4], f32, tag="q", name="q")[:, :m_tiles]
            qs = smpool.tile([P, 4], f32, tag="qs", name="qs")[:, :m_tiles]
            f = smpool.tile([P, 4], f32, tag="f", name="f")[:, :m_tiles]
            for m in range(m_tiles):
                po = pso.tile([P, D], f32, tag="po")
                if lo:
                    lhs_src = hts[0][:, :, m * P:(m + 1) * P]
                    for hp in range(NH // 2):
                        for n2 in range(2):
                            nc.tensor.matmul(
                                po[:, n2 * 512:(n2 + 1) * 512],
                                lhsT=lhs_src[:, 2 * hp:2 * hp + 2, :],
                                rhs=w28_sb[:, 2 * hp:2 * hp + 2,
                                           n2 * 512:(n2 + 1) * 512],
                                start=(hp == 0), stop=(hp == NH // 2 - 1),
                                perf_mode=DR)
                else:
                    ht = hts[m // 2]
                    o = (m % 2) * P
                    for h in range(NH):
                        for n2 in range(2):
                            nc.tensor.matmul(
                                po[:, n2 * 512:(n2 + 1) * 512],
                                lhsT=ht[:, h, o:o + P],
                                rhs=w2_sb[:, h, n2 * 512:(n2 + 1) * 512],
                                start=(h == 0), stop=(h == NH - 1))
                stage = stpool.tile([P, D], f32, tag="stage", name="stage")
                if has_b2:
                    nc.vector.tensor_add(out=stage, in0=po,
                                         in1=(b2s_sb if lo else b2_sb))
                    src = stage
                else:
                    src = po
                sq = sqpool.tile([P, D], f32, tag="sq")
                nc.scalar.activation(out=sq, in_=src, func=AF.Square,
                                     accum_out=q[:, m:m + 1])
                nc.scalar.activation(out=qs[:, m:m + 1], in_=q[:, m:m + 1],
                                     func=AF.Sqrt)
                nc.vector.reciprocal(out=qs[:, m:m + 1], in_=qs[:, m:m + 1])
                nc.vector.tensor_mul(out=f[:, m:m + 1], in0=qs[:, m:m + 1],
                                     in1=sc_sb[:, B // P + m:B // P + m + 1])
                nc.vector.tensor_scalar_mul(out=stage, in0=src,
                                            scalar1=f[:, m:m + 1])
                last = bi == n_blk - 1 and m == m_tiles - 1
                eng = nc.sync if last else nc.gpsimd
                eng.dma_start(out=y_t[:, B // P + m, :], in_=stage)

    nc.compile()
    return nc


def _get_nc(has_b2):
    key = ("nc", has_b2)
    if key not in _nc_cache:
        _nc_cache[key] = _build_nc(has_b2)
    return _nc_cache[key]


LAST_EXEC_NS = None
LAST_TRACE = None


def _install_axon_ntff_hook():
    """Register antenv.axon_hooks shim driving NTFF capture via the axon .so."""
    import contextlib
    import ctypes
    import types

    if "antenv.axon_hooks" in sys.modules:
        return
    lib = ctypes.CDLL("/opt/axon/libaxon_pjrt.so")
    if not hasattr(lib, "axon_start_nrt_profile"):
        return
    lib.axon_start_nrt_profile.argtypes = [ctypes.POINTER(ctypes.c_int64),
                                           ctypes.c_size_t]
    lib.axon_start_nrt_profile.restype = ctypes.c_int64
    lib.axon_stop_nrt_profile.argtypes = [ctypes.c_char_p]
    lib.axon_stop_nrt_profile.restype = ctypes.c_int64

    @contextlib.contextmanager
    def _hook(output_dir, device_ids):
        import jax
        jax.devices()
        if device_ids:
            ids = (ctypes.c_int64 * len(device_ids))(*device_ids)
            rc = lib.axon_start_nrt_profile(ids, len(device_ids))
        else:
            rc = lib.axon_start_nrt_profile(None, 0)
        if rc != 0:
            raise RuntimeError(f"axon_start_nrt_profile rc={rc}")
        try:
            yield
        finally:
            n = lib.axon_stop_nrt_profile(str(output_dir).encode())
            print(f"ntff capture: {n} file(s) -> {output_dir}", file=sys.stderr)

    mod = types.ModuleType("antenv.axon_hooks")
    mod.get_axon_ntff_profile_hook = lambda: _hook
    sys.modules["antenv.axon_hooks"] = mod
    import antenv
    antenv.axon_hooks = mod


def _gating(x, w_gate, k):
    """Top-k gating computed exactly like the reference (CPU jax, fp32)."""
    import jax
    import jax.numpy as jnp

    cpu = jax.devices("cpu")[0]
    with jax.default_device(cpu):
        xj = jnp.asarray(x)
        logits = xj @ jnp.asarray(w_gate)
        top_vals, top_idx = jax.lax.top_k(logits, k)
        top_gates = jax.nn.softmax(top_vals, axis=-1)
        init_norm = jnp.linalg.norm(xj, axis=-1)
        return (np.asarray(top_idx), np.asarray(top_gates, np.float32),
                np.asarray(init_norm, np.float32))


def kernel(x, w_gate, w1, b1, w2, b2, k):
    from concourse.bass_utils import run_bass_kernel_spmd

    x = np.asarray(x, np.float32)
    w_gate = np.asarray(w_gate, np.float32)
    w1 = np.asarray(w1, np.float32)
    b1 = np.asarray(b1, np.float32)
    w2 = np.asarray(w2, np.float32)
    b2 = np.asarray(b2, np.float32)
    k = int(np.asarray(k))
    n, d = x.shape
    e = w_gate.shape[1]

    top_idx, top_gates, init_norm = _gating(x, w_gate, k)

    idxs, scs = [], []
    for ei in range(e):
        tok, slot = np.nonzero(top_idx == ei)
        w = top_gates[tok, slot] * init_norm[tok]
        order = np.argsort(-w)
        assert len(tok) <= C_TOT, f"expert {ei} load {len(tok)} > {C_TOT}"
        idxs.append(tok[order])
        scs.append(w[order])

    has_b2 = bool(np.any(b2))
    nc = _get_nc(has_b2)

    in_maps = []
    for ei in range(e):
        tok = idxs[ei]
        n_hi = min(len(tok), C_HI)
        xg_hi = np.zeros((C_HI, d), np.float32)
        xg_hi[:n_hi] = x[tok[:n_hi]]
        xg_lo = np.zeros((C_LO, d), np.float32)
        xg_lo[:len(tok) - n_hi] = x[tok[n_hi:]]
        sce = np.zeros((C_TOT,), np.float32)
        sce[:n_hi] = scs[ei][:n_hi]
        sce[C_HI:C_HI + len(tok) - n_hi] = scs[ei][n_hi:]
        sce = np.ascontiguousarray(sce.reshape(C_TOT // P, P).T)
        in_maps.append({
            "xT": _tile_xT(xg_hi, HI_BLOCKS, BF16),
            "xT8": _tile_xT(_q8(xg_lo, S_X).astype(np.float32), LO_BLOCKS,
                            np.float32).astype(F8),
            "w1": _tile_w1(w1[ei], BF16),
            "w18": _tile_w1(_q8(w1[ei], S_W1).astype(np.float32),
                            np.float32).astype(F8),
            "b1": np.ascontiguousarray(b1[ei].reshape(NH, P).T),
            "b1s": np.ascontiguousarray((S_H * b1[ei]).reshape(NH, P).T),
            "w2": _tile_w2(w2[ei], BF16),
            "w28": _tile_w2(_q8(w2[ei], S_W2).astype(np.float32),
                            np.float32).astype(F8),
            "b2": np.ascontiguousarray(b2[ei]),
            "b2s": np.ascontiguousarray(S_OUT * b2[ei]),
            "sc": sce,
        })

    trace = bool(int(os.environ.get("MOE_TRACE", "0")))
    kwargs = {}
    if trace:
        _install_axon_ntff_hook()
        tdir = os.environ.get("MOE_TRACE_DIR")
        if tdir:
            os.makedirs(tdir, exist_ok=True)
            kwargs["tmpdir"] = tdir
        kwargs["trace_cores"] = [0]
    res = run_bass_kernel_spmd(
        nc, in_maps, core_ids=list(range(e)), trace=trace, **kwargs,
    )
    global LAST_EXEC_NS, LAST_TRACE
    LAST_EXEC_NS = res.exec_time_ns
    LAST_TRACE = res.instructions_and_trace
    if res.exec_time_ns is not None:
        print(f"HW exec time: {res.exec_time_ns} ns", file=sys.stderr)

    y = np.zeros((n, d), np.float32)
    for ei in range(e):
        tok = idxs[ei]
        n_hi = min(len(tok), C_HI)
        ydev = res.results[ei]["y"]
        y[tok[:n_hi]] += ydev[:n_hi]
        y[tok[n_hi:]] += ydev[C_HI:C_HI + len(tok) - n_hi]
    return y


# revision 14
# speedup vs baseline: 1.1491x; 1.0012x over previous
"""Expert-parallel mixed-precision MoE kernel for Trainium2 (8 NeuronCores).

Problem: top-2 MoE, N=8192 tokens, D=1024, H=4096, E=8 experts.
Strategy (expert parallel, per-token mixed precision):
  - Host: compute gating (logits -> top-k -> softmax) exactly as the
    reference does (CPU jax, fp32), dispatch tokens to their experts.
  - Core e holds expert e's tokens, split into two tiers by combine
    weight g*||x||: the top C_HI=1664 run a bf16 pipeline, the rest
    (<=C_LO=512) run an fp8-e4m3 DoubleRow pipeline (2x matmul rate,
    measured 216ns for a 256-deep x 512-wide MM).  Output combine
    rescale:  y = mlp(x) * (gate * ||x||) / ||mlp(x)||.
    fp8 error only hits low-gate tokens; exact rel err ~1.5e-2.
  - Host: scatter-add per-expert outputs back to the [N, D] result.

Device kernel (per core, fp32 PSUM accumulation):
  Token blocks: hi [512, 384, 384, 384] bf16, lo [512] fp8.  Layer 1
  computes hT [H, R] (H on partitions) by streaming w1 per h-tile,
  relu-ing into two token-half hT tiles (3-buffer pool, so block n+1's
  layer 1 never serializes against block n's layer 2). Layer 2
  accumulates out[R, D] in PSUM over h with w2 resident in SBUF (bf16
  64KB + fp8 32KB per partition).  Per-m-tile epilogue straight from
  PSUM (b2 is zero): ACT Square with accum -> q, ACT Rsqrt, DVE mul
  by sc, DVE scale, DMA out — so y DMAs spread through layer 2 and
  the kernel tail only flushes one 512KB transfer.
  fp8 scales (x*16, w1*1024, h*32, w2*1024) fold into the relu
  scale/bias; the final y is exact fp32 (stage*f cancels the 32768x
  product scale; padded rows hit rsqrt(0)=inf but are never read).

  DMA: block-0 xt is loaded in four 2-k-tile chunks interleaved with
  the first h-group's matmuls; w2/w2_f8 ride the sync HWDGE FIFO
  paced between w1 chunks of the first two blocks; y outputs go on
  the gpsimd SWDGE queue except the last m-tile (sync).
"""

import os
import sys

import numpy as np

if "/opt/trn_rl_repo" not in sys.path:
    sys.path.insert(0, "/opt/trn_rl_repo")

import ml_dtypes

N, D, H, E = 8192, 1024, 4096, 8
P = 128
NK = D // P   # 8
NH = H // P   # 32
BF16 = ml_dtypes.bfloat16
F8 = ml_dtypes.float8_e4m3

C_HI, C_LO = 1664, 512
HI_BLOCKS = [512, 384, 384, 384]
LO_BLOCKS = [512]
C_TOT = C_HI + C_LO
S_X, S_W1, S_H, S_W2 = 16.0, 1024.0, 32.0, 1024.0
S_L1 = S_X * S_W1            # layer-1 psum scale
S_OUT = S_H * S_W2           # layer-2 psum scale

assert sum(HI_BLOCKS) == C_HI and sum(LO_BLOCKS) == C_LO

_nc_cache = {}


def _tile_w1(w1e, dt):
    """[D, H] fp32 -> [P, NH, NK, P] with w1t[p,h,k,j] = w1e[k*P+p, h*P+j]."""
    return np.ascontiguousarray(
        w1e.reshape(NK, P, NH, P).transpose(1, 2, 0, 3).astype(dt))


def _tile_w2(w2e, dt):
    """[H, D] fp32 -> [P, NH, D] with w2t[p,h,d] = w2e[h*P+p, d]."""
    return np.ascontiguousarray(
        w2e.reshape(NH, P, D).transpose(1, 0, 2).astype(dt))


def _tile_xT(xg, blocks, dt):
    """[C, D] fp32 (padded) -> [P, NK*C], per-block [k, j] segments."""
    C = xg.shape[0]
    out = np.zeros((P, NK * C), dt)
    B = 0
    for R in blocks:
        seg = xg[B:B + R].T.reshape(NK, P, R).transpose(1, 0, 2)
        out[:, NK * B:NK * (B + R)] = seg.reshape(P, NK * R)
        B += R
    return out


def _q8(a, scale):
    return np.clip(np.asarray(a, np.float32) * scale, -240, 240).astype(F8)


def _build_nc(has_b2):
    from contextlib import ExitStack

    import concourse.bass as bass
    import concourse.mybir as mybir
    import concourse.tile as tile
    from concourse import bacc

    f32 = mybir.dt.float32
    bf16 = mybir.dt.bfloat16
    f8 = mybir.dt.float8e4
    AF = mybir.ActivationFunctionType
    DR = mybir.MatmulPerfMode.DoubleRow

    nc = bacc.Bacc(trn_type="TRN2", num_devices=E)
    xT = nc.dram_tensor("xT", [P, NK * C_HI], bf16, kind="ExternalInput")
    xT8 = nc.dram_tensor("xT8", [P, NK * C_LO], f8, kind="ExternalInput")
    w1 = nc.dram_tensor("w1", [P, NH, NK, P], bf16, kind="ExternalInput")
    w18 = nc.dram_tensor("w18", [P, NH, NK, P], f8, kind="ExternalInput")
    b1 = nc.dram_tensor("b1", [P, NH], f32, kind="ExternalInput")
    b1s = nc.dram_tensor("b1s", [P, NH], f32, kind="ExternalInput")
    w2 = nc.dram_tensor("w2", [P, NH, D], bf16, kind="ExternalInput")
    w28 = nc.dram_tensor("w28", [P, NH, D], f8, kind="ExternalInput")
    b2 = nc.dram_tensor("b2", [D], f32, kind="ExternalInput")
    b2s = nc.dram_tensor("b2s", [D], f32, kind="ExternalInput")
    sc = nc.dram_tensor("sc", [P, C_TOT // P], f32, kind="ExternalInput")
    y = nc.dram_tensor("y", [C_TOT, D], f32, kind="ExternalOutput")

    y_t = y.ap().rearrange("(o p) d -> p o d", p=P)

    with tile.TileContext(nc) as tc, ExitStack() as ctx:
        singles = ctx.enter_context(tc.tile_pool(name="singles", bufs=1))
        xpool = ctx.enter_context(tc.tile_pool(name="xpool", bufs=2))
        x0pool = ctx.enter_context(tc.tile_pool(name="x0pool", bufs=1))
        w1pool = ctx.enter_context(tc.tile_pool(name="w1pool", bufs=5))
        hpool = ctx.enter_context(tc.tile_pool(name="hpool", bufs=3))
        stpool = ctx.enter_context(tc.tile_pool(name="stpool", bufs=2))
        sqpool = ctx.enter_context(tc.tile_pool(name="sqpool", bufs=1))
        smpool = ctx.enter_context(tc.tile_pool(name="smpool", bufs=4))
        psh = ctx.enter_context(tc.tile_pool(name="psh", bufs=2, space="PSUM"))
        pso = ctx.enter_context(tc.tile_pool(name="pso", bufs=3, space="PSUM"))

        # --- constants (gpsimd SWDGE queue; small) ---
        b1_sb = singles.tile([P, NH], f32)
        nc.gpsimd.dma_start(out=b1_sb, in_=b1.ap())
        b1s_sb = singles.tile([P, NH], f32)
        nc.gpsimd.dma_start(out=b1s_sb, in_=b1s.ap())
        sc_sb = singles.tile([P, C_TOT // P], f32)
        nc.gpsimd.dma_start(out=sc_sb, in_=sc.ap())
        if has_b2:
            b2_sb = singles.tile([P, D], f32)
            b2_bcast = bass.AP(tensor=b2.ap().tensor, offset=b2.ap().offset,
                               ap=[[0, P], *b2.ap().ap])
            nc.gpsimd.dma_start(out=b2_sb, in_=b2_bcast)
            b2s_sb = singles.tile([P, D], f32)
            b2s_bcast = bass.AP(tensor=b2s.ap().tensor, offset=b2s.ap().offset,
                                ap=[[0, P], *b2s.ap().ap])
            nc.gpsimd.dma_start(out=b2s_sb, in_=b2s_bcast)
        # w2 / w2_f8 are paced between w1 chunks on the sync FIFO below.
        w2_sb = singles.tile([P, NH, D], bf16)
        w28_sb = singles.tile([P, NH, D], f8)

        blocks = ([("hi", B, R) for B, R in
                   zip(np.cumsum([0] + HI_BLOCKS[:-1]).tolist(), HI_BLOCKS)]
                  + [("lo", B, R) for B, R in
                     zip((C_HI + np.cumsum([0] + LO_BLOCKS[:-1])).tolist(),
                         LO_BLOCKS)])
        n_blk = len(blocks)

        def make_xt(bj, eng):
            """Allocate block bj's x tile; DMA it on `eng` (bulk prefetch)."""
            tier_j, B_j, R_j = blocks[bj]
            if tier_j == "lo":
                t = xpool.tile([P, NK, 512], f8, tag="xt", name="xt8")[:, :, :R_j]
                eng.dma_start(
                    out=t,
                    in_=xT8.ap()[:, NK * (B_j - C_HI):NK * (B_j - C_HI + R_j)]
                    .rearrange("p (k j) -> p k j", k=NK))
            else:
                t = xpool.tile([P, NK, 512], bf16, tag="xt", name="xt")[:, :, :R_j]
                eng.dma_start(
                    out=t,
                    in_=xT.ap()[:, NK * B_j:NK * (B_j + R_j)]
                    .rearrange("p (k j) -> p k j", k=NK))
            return t

        xts = {}
        for bi, (tier, B, R) in enumerate(blocks):
            m_tiles = (R + P - 1) // P
            lo = tier == "lo"
            # last hi block's b-half (the lowest-weight bf16-L1 tokens) gets
            # an fp8 layer 2: relu casts it to fp8 at the lo scales.
            mid = (not lo) and bi == len(HI_BLOCKS) - 1
            first = bi == 0
            Ra = min(R, 256)          # token-half split for hi hT tiles

            if first:
                # block 0: chunked loads in four separate tiles (whole-tile
                # DMA dep granularity: one tile would stall MM#1 on chunk 4)
                xchunks = [x0pool.tile([P, 2, 512], bf16, tag=f"x0_{c}",
                                       name=f"x0_{c}") for c in range(4)]
            else:
                xt = xts.pop(bi)

            # --- layer 1: hT[h, tok] (H on partitions) ---
            if lo:
                hts = [hpool.tile([P, NH, 512], f8, tag="hT",
                                  name=f"hT{bi}")[:, :, :R]]
            else:
                hts = [hpool.tile([P, NH, 256], bf16, tag="hT",
                                  name=f"hTa{bi}")[:, :, :Ra],
                       hpool.tile([P, NH, 256], f8 if mid else bf16, tag="hT",
                                  name=f"hTb{bi}")[:, :, :R - Ra]]
            for h in range(NH):
                w1c = w1pool.tile([P, NK, P], f8 if lo else bf16,
                                  tag="w1c", name=f"w1c{bi}_{h}")
                nc.sync.dma_start(out=w1c,
                                  in_=(w18 if lo else w1).ap()[:, h])
                # Bulk prefetch rides the scalar-engine HWDGE queue: the
                # SDMA engines round-robin queues at packet granularity,
                # so these big streams don't starve the w1 JIT chunks.
                if h == 2 and bi + 1 < n_blk:
                    xts[bi + 1] = make_xt(bi + 1, nc.scalar)
                if bi == 0 and h % 4 == 2:
                    nc.scalar.dma_start(out=w2_sb[:, h - 2:h + 2, :],
                                        in_=w2.ap()[:, h - 2:h + 2, :])
                if bi == 1 and h % 8 == 2:
                    nc.scalar.dma_start(out=w28_sb[:, h - 2:h + 6, :],
                                        in_=w28.ap()[:, h - 2:h + 6, :])
                ps = psh.tile([P, 512], f32, tag="ph", name="ph")[:, :R]
                if first and h == 0:
                    for c in range(4):
                        nc.sync.dma_start(
                            out=xchunks[c],
                            in_=xT.ap()[:, NK * B + 2 * c * 512:
                                        NK * B + (2 * c + 2) * 512]
                            .rearrange("p (k j) -> p k j", k=2))
                        for k2 in range(2):
                            nc.tensor.matmul(
                                ps, lhsT=w1c[:, 2 * c + k2, :],
                                rhs=xchunks[c][:, k2, :],
                                start=(c == 0 and k2 == 0),
                                stop=(c == 3 and k2 == 1))
                elif first:
                    for c in range(4):
                        for k2 in range(2):
                            nc.tensor.matmul(
                                ps, lhsT=w1c[:, 2 * c + k2, :],
                                rhs=xchunks[c][:, k2, :],
                                start=(c == 0 and k2 == 0),
                                stop=(c == 3 and k2 == 1))
                elif lo:
                    for kp in range(NK // 2):
                        nc.tensor.matmul(
                            ps, lhsT=w1c[:, 2 * kp:2 * kp + 2, :],
                            rhs=xt[:, 2 * kp:2 * kp + 2, :],
                            start=(kp == 0), stop=(kp == NK // 2 - 1),
                            perf_mode=DR)
                else:
                    for k in range(NK):
                        nc.tensor.matmul(
                            ps, lhsT=w1c[:, k, :], rhs=xt[:, k, :],
                            start=(k == 0), stop=(k == NK - 1))
                if lo:
                    nc.scalar.activation(
                        out=hts[0][:, h, :], in_=ps, func=AF.Relu,
                        bias=b1s_sb[:, h:h + 1], scale=S_H / S_L1)
                else:
                    nc.scalar.activation(
                        out=hts[0][:, h, :], in_=ps[:, :Ra], func=AF.Relu,
                        bias=b1_sb[:, h:h + 1], scale=1.0)
                    nc.scalar.activation(
                        out=hts[1][:, h, :], in_=ps[:, Ra:], func=AF.Relu,
                        bias=(b1s_sb if mid else b1_sb)[:, h:h + 1],
                        scale=S_H if mid else 1.0)

            # --- layer 2 + per-m-tile epilogue ---
            q = smpool.tile([P, 4], f32, tag="q", name="q")[:, :m_tiles]
            qs = smpool.tile([P, 4], f32, tag="qs", name="qs")[:, :m_tiles]
            f = smpool.tile([P, 4], f32, tag="f", name="f")[:, :m_tiles]
            for m in range(m_tiles):
                po = pso.tile([P, D], f32, tag="po")
                if lo or (mid and m // 2 == 1):
                    lhs_src = (hts[0][:, :, m * P:(m + 1) * P] if lo
                               else hts[1][:, :, (m % 2) * P:(m % 2) * P + P])
                    for hp in range(NH // 2):
                        for n2 in range(2):
                            nc.tensor.matmul(
                                po[:, n2 * 512:(n2 + 1) * 512],
                                lhsT=lhs_src[:, 2 * hp:2 * hp + 2, :],
                                rhs=w28_sb[:, 2 * hp:2 * hp + 2,
                                           n2 * 512:(n2 + 1) * 512],
                                start=(hp == 0), stop=(hp == NH // 2 - 1),
                                perf_mode=DR)
                else:
                    ht = hts[m // 2]
                    o = (m % 2) * P
                    for h in range(NH):
                        for n2 in range(2):
                            nc.tensor.matmul(
                                po[:, n2 * 512:(n2 + 1) * 512],
                                lhsT=ht[:, h, o:o + P],
                                rhs=w2_sb[:, h, n2 * 512:(n2 + 1) * 512],
                                start=(h == 0), stop=(h == NH - 1))
                stage = stpool.tile([P, D], f32, tag="stage", name="stage")
                if has_b2:
                    nc.vector.tensor_add(out=stage, in0=po,
                                         in1=(b2s_sb if lo else b2_sb))
                    src = stage
                else:
                    src = po
                sq = sqpool.tile([P, D], f32, tag="sq")
                nc.scalar.activation(out=sq, in_=src, func=AF.Square,
                                     accum_out=q[:, m:m + 1])
                nc.scalar.activation(out=qs[:, m:m + 1], in_=q[:, m:m + 1],
                                     func=AF.Sqrt)
                nc.vector.reciprocal(out=qs[:, m:m + 1], in_=qs[:, m:m + 1])
                nc.vector.tensor_mul(out=f[:, m:m + 1], in0=qs[:, m:m + 1],
                                     in1=sc_sb[:, B // P + m:B // P + m + 1])
                nc.vector.tensor_scalar_mul(out=stage, in0=src,
                                            scalar1=f[:, m:m + 1])
                last = bi == n_blk - 1 and m == m_tiles - 1
                eng = nc.sync if last else nc.gpsimd
                eng.dma_start(out=y_t[:, B // P + m, :], in_=stage)

    nc.compile()
    return nc


def _get_nc(has_b2):
    key = ("nc", has_b2)
    if key not in _nc_cache:
        _nc_cache[key] = _build_nc(has_b2)
    return _nc_cache[key]


LAST_EXEC_NS = None
LAST_TRACE = None


def _install_axon_ntff_hook():
    """Register antenv.axon_hooks shim driving NTFF capture via the axon .so."""
    import contextlib
    import ctypes
    import types

    if "antenv.axon_hooks" in sys.modules:
        return
    lib = ctypes.CDLL("/opt/axon/libaxon_pjrt.so")
    if not hasattr(lib, "axon_start_nrt_profile"):
        return
    lib.axon_start_nrt_profile.argtypes = [ctypes.POINTER(ctypes.c_int64),
                                           ctypes.c_size_t]
    lib.axon_start_nrt_profile.restype = ctypes.c_int64
    lib.axon_stop_nrt_profile.argtypes = [ctypes.c_char_p]
    lib.axon_stop_nrt_profile.restype = ctypes.c_int64

    @contextlib.contextmanager
    def _hook(output_dir, device_ids):
        import jax
        jax.devices()
        if device_ids:
            ids = (ctypes.c_int64 * len(device_ids))(*device_ids)
            rc = lib.axon_start_nrt_profile(ids, len(device_ids))
        else:
            rc = lib.axon_start_nrt_profile(None, 0)
        if rc != 0:
            raise RuntimeError(f"axon_start_nrt_profile rc={rc}")
        try:
            yield
        finally:
            n = lib.axon_stop_nrt_profile(str(output_dir).encode())
            print(f"ntff capture: {n} file(s) -> {output_dir}", file=sys.stderr)

    mod = types.ModuleType("antenv.axon_hooks")
    mod.get_axon_ntff_profile_hook = lambda: _hook
    sys.modules["antenv.axon_hooks"] = mod
    import antenv
    antenv.axon_hooks = mod


def _gating(x, w_gate, k):
    """Top-k gating computed exactly like the reference (CPU jax, fp32)."""
    import jax
    import jax.numpy as jnp

    cpu = jax.devices("cpu")[0]
    with jax.default_device(cpu):
        xj = jnp.asarray(x)
        logits = xj @ jnp.asarray(w_gate)
        top_vals, top_idx = jax.lax.top_k(logits, k)
        top_gates = jax.nn.softmax(top_vals, axis=-1)
        init_norm = jnp.linalg.norm(xj, axis=-1)
        return (np.asarray(top_idx), np.asarray(top_gates, np.float32),
                np.asarray(init_norm, np.float32))


def kernel(x, w_gate, w1, b1, w2, b2, k):
    from concourse.bass_utils import run_bass_kernel_spmd

    x = np.asarray(x, np.float32)
    w_gate = np.asarray(w_gate, np.float32)
    w1 = np.asarray(w1, np.float32)
    b1 = np.asarray(b1, np.float32)
    w2 = np.asarray(w2, np.float32)
    b2 = np.asarray(b2, np.float32)
    k = int(np.asarray(k))
    n, d = x.shape
    e = w_gate.shape[1]

    top_idx, top_gates, init_norm = _gating(x, w_gate, k)

    idxs, scs = [], []
    for ei in range(e):
        tok, slot = np.nonzero(top_idx == ei)
        w = top_gates[tok, slot] * init_norm[tok]
        order = np.argsort(-w)
        assert len(tok) <= C_TOT, f"expert {ei} load {len(tok)} > {C_TOT}"
        idxs.append(tok[order])
        scs.append(w[order])

    has_b2 = bool(np.any(b2))
    nc = _get_nc(has_b2)

    in_maps = []
    for ei in range(e):
        tok = idxs[ei]
        n_hi = min(len(tok), C_HI)
        xg_hi = np.zeros((C_HI, d), np.float32)
        xg_hi[:n_hi] = x[tok[:n_hi]]
        xg_lo = np.zeros((C_LO, d), np.float32)
        xg_lo[:len(tok) - n_hi] = x[tok[n_hi:]]
        sce = np.zeros((C_TOT,), np.float32)
        sce[:n_hi] = scs[ei][:n_hi]
        sce[C_HI:C_HI + len(tok) - n_hi] = scs[ei][n_hi:]
        sce = np.ascontiguousarray(sce.reshape(C_TOT // P, P).T)
        in_maps.append({
            "xT": _tile_xT(xg_hi, HI_BLOCKS, BF16),
            "xT8": _tile_xT(_q8(xg_lo, S_X).astype(np.float32), LO_BLOCKS,
                            np.float32).astype(F8),
            "w1": _tile_w1(w1[ei], BF16),
            "w18": _tile_w1(_q8(w1[ei], S_W1).astype(np.float32),
                            np.float32).astype(F8),
            "b1": np.ascontiguousarray(b1[ei].reshape(NH, P).T),
            "b1s": np.ascontiguousarray((S_H * b1[ei]).reshape(NH, P).T),
            "w2": _tile_w2(w2[ei], BF16),
            "w28": _tile_w2(_q8(w2[ei], S_W2).astype(np.float32),
                            np.float32).astype(F8),
            "b2": np.ascontiguousarray(b2[ei]),
            "b2s": np.ascontiguousarray(S_OUT * b2[ei]),
            "sc": sce,
        })

    trace = bool(int(os.environ.get("MOE_TRACE", "0")))
    kwargs = {}
    if trace:
        _install_axon_ntff_hook()
        tdir = os.environ.get("MOE_TRACE_DIR")
        if tdir:
            os.makedirs(tdir, exist_ok=True)
            kwargs["tmpdir"] = tdir
        kwargs["trace_cores"] = [0]
    res = run_bass_kernel_spmd(
        nc, in_maps, core_ids=list(range(e)), trace=trace, **kwargs,
    )
    global LAST_EXEC_NS, LAST_TRACE
    LAST_EXEC_NS = res.exec_time_ns
    LAST_TRACE = res.instructions_and_trace
    if res.exec_time_ns is not None:
        print(f"HW exec time: {res.exec_time_ns} ns", file=sys.stderr)

    y = np.zeros((n, d), np.float32)
    for ei in range(e):
        tok = idxs[ei]
        n_hi = min(len(tok), C_HI)
        ydev = res.results[ei]["y"]
        y[tok[:n_hi]] += ydev[:n_hi]
        y[tok[n_hi:]] += ydev[C_HI:C_HI + len(tok) - n_hi]
    return y
